# revision 5
# baseline (speedup 1.0000x reference)
"""Trainium2 Bass kernel for nn_DecoderLayer (self-attn + cross-attn + FFN).

Sharding: 8 cores = 2 batches x 4 query-blocks of 512 tokens (data/sequence
parallel, zero collectives). Each core recomputes the full K/V for its batch
and runs its 512 queries through the whole layer.

v2: all matmul operands bf16 (weights/activations cast host-side where
possible) -- same 1 cyc/row PE rate as f32r but half the DMA bytes and half
the SBUF footprint. K/V for all 4 head-groups stay resident so X/enc stream
exactly once per attention. Per-attention order is K/V proj -> Q proj ->
heads, so cross-attention K/V (which needs only encT) overlaps LN1's vector
chain. Two heads sharing a kt dtile are interleaved per j-chunk so the PE
stays fed through the scalar-engine exp latency; residual adds + LayerNorm
statistics are deferred to one block after the head loop so the stat
matmuls never stall the tensor stream on a softmax-normalize chain.
Next-phase weights (CA, FFN slabs) and encoder blocks are prefetched
before each head loop.
"""

import sys

if "/opt/trn_rl_repo" not in sys.path:
    sys.path.insert(0, "/opt/trn_rl_repo")

import numpy as np

D = 1024
S = 2048
QTOK = 512          # queries per core
H = 16
HD = 64
FFN = 4096
EPS = 1e-5
NCORES = 8

_PROGRAM_CACHE = {}


def _build_program():
    import contextlib

    import concourse.bacc as bacc
    import concourse.tile as tile
    from concourse import mybir

    f32 = mybir.dt.float32
    bf = mybir.dt.bfloat16
    AF = mybir.ActivationFunctionType
    Alu = mybir.AluOpType

    nc = bacc.Bacc("TRN2", target_bir_lowering=False)

    def din(name, shape, dt=bf):
        return nc.declare_dram_parameter(name, list(shape), dt, isOutput=False)

    xT = din("xT", (128, 8, S))
    xqT = din("xqT", (128, 8, QTOK))
    encT = din("encT", (128, 8, S))
    w = {}
    for p in ("sa", "ca"):
        for wn in ("wq", "wk", "wv"):
            w[f"{p}_{wn}"] = din(f"{p}_{wn}", (128, 8, D))
        for bn in ("bq", "bk"):
            w[f"{p}_{bn}"] = din(f"{p}_{bn}", (128, 8), f32)
        w[f"{p}_bv"] = din(f"{p}_bv", (128, D))
    fc1_w = din("fc1_w", (128, 8, FFN))
    fc2_w = din("fc2_w", (128, 32, D))
    fc1_b = din("fc1_b", (128, 32), f32)
    fc2_b = din("fc2_b", (128, 8), f32)
    for i in (1, 2, 3):
        w[f"ln{i}_g"] = din(f"ln{i}_g", (128, 8), f32)
        w[f"ln{i}_b"] = din(f"ln{i}_b", (128, 8), f32)
    outT = nc.declare_dram_parameter("outT", [D, QTOK], f32, isOutput=True)
    outT_r = outT.rearrange("(i p) t -> p i t", p=128)

    with tile.TileContext(nc) as tc:
        with contextlib.ExitStack() as ctx:
            consts = ctx.enter_context(tc.tile_pool(name="consts", bufs=1))
            pw = ctx.enter_context(tc.tile_pool(name="pw", bufs=3))
            pkt = ctx.enter_context(tc.tile_pool(name="pkt", bufs=8))
            pvp = ctx.enter_context(tc.tile_pool(name="pvp", bufs=1))
            pqt = ctx.enter_context(tc.tile_pool(name="pqt", bufs=1))
            pxb = ctx.enter_context(tc.tile_pool(name="pxb", bufs=2))
            pctx = ctx.enter_context(tc.tile_pool(name="pctx", bufs=1))
            pxa = ctx.enter_context(tc.tile_pool(name="pxa", bufs=1))
            ph2 = ctx.enter_context(tc.tile_pool(name="ph2", bufs=1))
            pst = ctx.enter_context(tc.tile_pool(name="pst", bufs=1))
            psq = ctx.enter_context(tc.tile_pool(name="psq", bufs=2))
            pexp = ctx.enter_context(tc.tile_pool(name="pexp", bufs=4))
            pout = ctx.enter_context(tc.tile_pool(name="pout", bufs=1))
            ppp = ctx.enter_context(tc.tile_pool(name="ppp", bufs=2, space="PSUM"))
            pps = ctx.enter_context(tc.tile_pool(name="pps", bufs=2, space="PSUM"))
            ppc = ctx.enter_context(tc.tile_pool(name="ppc", bufs=2, space="PSUM"))

            # ---- constants ----
            ones = consts.tile([128, 1], bf, tag="ones")
            nc.vector.memset(ones, 1.0)
            xq_sb = consts.tile([128, 8, QTOK], bf, tag="xq")
            nc.gpsimd.dma_start(out=xq_sb, in_=xqT[:, :, :])
            sb = {}
            for name, hnd in w.items():
                if name.endswith(("wq", "wk", "wv")):
                    continue
                dt_ = bf if name.endswith("_bv") else f32
                sb[name] = consts.tile(
                    [128, hnd.shape[1]], dt_, tag=name, name=name
                )
                nc.scalar.dma_start(out=sb[name], in_=hnd[:, :])
            fc1b_sb = consts.tile([128, 32], f32, tag="fc1b")
            nc.scalar.dma_start(out=fc1b_sb, in_=fc1_b[:, :])
            fc2b_sb = consts.tile([128, 8], f32, tag="fc2b")
            nc.scalar.dma_start(out=fc2b_sb, in_=fc2_b[:, :])

            def attention(pfx, src_dram, q_sb, ctx_tile, resid,
                          prefetch=None, pre_xb=None, pre_w=None, mid=None):
                """One MHA. K/V proj (streaming src once) -> Q proj -> heads.
                After each head pair, the residual add + LN stat matmuls for
                that dtile are emitted (accumulating into sum_ps/sq_ps, which
                are allocated last on the 'acc' ring so nothing displaces
                them mid-accumulation). Returns (sum_ps, sq_ps)."""
                bq_sb, bk_sb, bv_sb = sb[f"{pfx}_bq"], sb[f"{pfx}_bk"], sb[f"{pfx}_bv"]
                if pre_w is not None:
                    wk, wv, wq = pre_w["wk"], pre_w["wv"], pre_w["wq"]
                else:
                    wk = pw.tile([128, 8, D], bf, tag="w", name="wk")
                    nc.sync.dma_start(out=wk, in_=w[f"{pfx}_wk"][:, :, :])
                    wv = pw.tile([128, 8, D], bf, tag="w", name="wv")
                    nc.sync.dma_start(out=wv, in_=w[f"{pfx}_wv"][:, :, :])
                    wq = pw.tile([128, 8, D], bf, tag="w", name="wq")
                    nc.sync.dma_start(out=wq, in_=w[f"{pfx}_wq"][:, :, :])

                kts = [
                    pkt.tile([128, S], bf, tag="kt", name=f"kt{_d}")
                    for _d in range(8)
                ]
                vp = pvp.tile([128, 16, 4, 261], bf, tag="vp", name="vp")
                # ones at col h*65 of each group block, plus trailing col 260
                nc.vector.memset(
                    vp[:, :, :, 0 : 65 * 4].rearrange(
                        "p j g (h u) -> p j g h u", u=65
                    )[:, :, :, :, 0:1],
                    1.0,
                )
                nc.vector.memset(vp[:, :, :, 260:261], 1.0)

                # ---- K/V projections, one pass over src ----
                for blk in range(4):
                    if pre_xb is not None and blk < len(pre_xb):
                        xb = pre_xb[blk]
                    else:
                        xb = pxb.tile([128, 8, QTOK], bf, tag="xb", name="xb")
                        nc.gpsimd.dma_start(
                            out=xb,
                            in_=src_dram[:, :, blk * 512 : (blk + 1) * 512],
                        )
                    for dt in range(8):
                        kacc = pps.tile([128, QTOK], f32, tag="ps", name="kacc")
                        for c in range(8):
                            nc.tensor.matmul(
                                kacc[:, :],
                                wk[:, c, dt * 128 : (dt + 1) * 128],
                                xb[:, c, :],
                                start=(c == 0),
                                stop=(c == 7),
                            )
                        nc.vector.tensor_scalar_add(
                            kts[dt][:, blk * 512 : (blk + 1) * 512],
                            kacc[:, :],
                            bk_sb[:, dt : dt + 1],
                        )
                    for tt in range(4):
                        j = blk * 4 + tt
                        for hf in range(2):
                            vacc = pps.tile([128, QTOK], f32, tag="ps",
                                            name="vacc")
                            for c in range(8):
                                nc.tensor.matmul(
                                    vacc[:, :],
                                    xb[:, c, tt * 128 : (tt + 1) * 128],
                                    wv[:, c, hf * 512 : (hf + 1) * 512],
                                    start=(c == 0),
                                    stop=(c == 7),
                                )
                            vc = psq.tile([128, QTOK], bf, tag="sq",
                                          name="vc")
                            nc.vector.tensor_copy(out=vc, in_=vacc[:, :])
                            for gg in range(2):
                                g = hf * 2 + gg
                                dst = vp[:, j, g, 1:261].rearrange(
                                    "p (h u) -> p h u", u=65
                                )[:, :, 0:64]
                                nc.vector.tensor_tensor(
                                    out=dst,
                                    in0=vc[:, gg * 256 : (gg + 1) * 256]
                                    .rearrange("p (h u) -> p h u", u=64),
                                    in1=bv_sb[:, g * 256 : (g + 1) * 256]
                                    .rearrange("p (h u) -> p h u", u=64),
                                    op=Alu.add,
                                )

                if mid is not None:
                    mid()
                # ---- Q projection ----
                qt = pqt.tile([128, 8, QTOK], bf, tag="qt", name="qt")
                for dt in range(8):
                    qacc = pps.tile([128, QTOK], f32, tag="ps", name="qacc")
                    for c in range(8):
                        nc.tensor.matmul(
                            qacc[:, :],
                            wq[:, c, dt * 128 : (dt + 1) * 128],
                            q_sb[:, c, :],
                            start=(c == 0),
                            stop=(c == 7),
                        )
                    nc.vector.tensor_scalar_add(
                        qt[:, dt, :], qacc[:, :], bq_sb[:, dt : dt + 1]
                    )

                if prefetch is not None:
                    prefetch()
                # ---- heads (+ interleaved LN statistic accumulation) ----
                # two heads (sharing a kt dtile) are interleaved per j-chunk
                # so the PE always has ~8 matmuls queued per exp window and
                # never idles into a lower p-state waiting on the scalar exp.
                sum_ps = ppp.tile([1, QTOK], f32, tag="acc", name="sum_ps")
                sq_ps = ppp.tile([1, QTOK], f32, tag="acc", name="sq_ps")
                for hp in range(8):
                    dt = hp
                    g = dt // 2
                    pcs = []
                    ets = [[], []]
                    for par in range(2):
                        pcs.append(ppc.tile([128, QTOK], f32, tag="pc",
                                            name=f"pc{par}"))
                    for jg in range(9):
                        if jg < 8:
                            for par in range(2):
                                ps = pps.tile([128, 2, QTOK], f32, tag="ps",
                                              name="ps")
                                for js in range(2):
                                    j = jg * 2 + js
                                    nc.tensor.matmul(
                                        ps[:, js, :],
                                        kts[dt][par * 64 : par * 64 + 64,
                                                j * 128 : (j + 1) * 128],
                                        qt[par * 64 : par * 64 + 64, dt, :],
                                        start=True,
                                        stop=True,
                                    )
                                et = pexp.tile([128, 2, QTOK], bf, tag="et",
                                               name="et")
                                nc.scalar.activation(
                                    out=et, in_=ps, func=AF.Exp,
                                    scale=1.0 / (HD ** 0.5),
                                )
                                ets[par].append(et)
                        if jg >= 1:
                            jj = jg - 1
                            for par in range(2):
                                h = 2 * hp + par
                                vlo = (h % 4) * 65 + 1
                                for js in range(2):
                                    j = jj * 2 + js
                                    nc.tensor.matmul(
                                        pcs[par][0:65, :],
                                        vp[:, j, g, vlo : vlo + 65],
                                        ets[par][jj][:, js, :],
                                        start=(jj == 0 and js == 0),
                                        stop=(jj == 7 and js == 1),
                                    )
                    # normalize both heads: reciprocal of denom (psum row
                    # 64), broadcast, multiply (DVE has no divide op)
                    for par in range(2):
                        pc = pcs[par]
                        dn = psq.tile([128, QTOK], f32, tag="sq", name="dn")
                        nc.vector.tensor_copy(out=dn[64:65, :],
                                              in_=pc[64:65, :])
                        nc.gpsimd.dma_start(out=dn[0:1, :], in_=dn[64:65, :])
                        dn2 = psq.tile([128, QTOK], f32, tag="sq",
                                       name="dn2")
                        nc.vector.reciprocal_approx_fast(out=dn2[0:1, :],
                                                         in_=dn[0:1, :])
                        db = pexp.tile([128, QTOK], f32, tag="db", name="db",
                                       bufs=2)
                        nc.gpsimd.partition_broadcast(
                            out_ap=db[0:64, :], in_ap=dn2[0:64, :], channels=64
                        )
                        if par == 0:
                            nc.vector.tensor_tensor(
                                out=ctx_tile[0:64, dt, :],
                                in0=pc[0:64, :],
                                in1=db[0:64, :],
                                op=Alu.mult,
                            )
                        else:
                            cn = psq.tile([128, QTOK], bf, tag="sq",
                                          name="cn")
                            nc.vector.tensor_tensor(
                                out=cn[0:64, :],
                                in0=pc[0:64, :],
                                in1=db[0:64, :],
                                op=Alu.mult,
                            )
                            nc.gpsimd.dma_start(
                                out=ctx_tile[64:128, dt, :], in_=cn[0:64, :]
                            )
                    # residual add + LN stats for this head-pair's dt,
                    # interleaved so stats complete with the last head and
                    # the LN chain can start immediately.
                    i = hp
                    nc.vector.tensor_tensor(
                        out=resid[:, i, :], in0=resid[:, i, :],
                        in1=ctx_tile[:, i, :], op=Alu.add,
                    )
                    nc.tensor.matmul(
                        sum_ps[:, :], ones[:, 0:1], resid[:, i, :],
                        start=(i == 0), stop=(i == 7),
                    )
                    sqt = psq.tile([128, QTOK], bf, tag="sq", name="sqt")
                    nc.vector.tensor_tensor(
                        out=sqt, in0=resid[:, i, :], in1=resid[:, i, :],
                        op=Alu.mult,
                    )
                    nc.tensor.matmul(
                        sq_ps[:, :], ones[:, 0:1], sqt[:, :],
                        start=(i == 0), stop=(i == 7),
                    )
                return sum_ps, sq_ps

            def ln_finish(sum_ps, sq_ps, gname, x_in, out_tile, out_dma=False):
                """Broadcast stats, then out = LN(x_in) * g + b per tile."""
                g_sb, b_sb = sb[f"{gname}_g"], sb[f"{gname}_b"]
                s_sb = pst.tile([128, 2, QTOK], f32, tag="sb", name="s_sb")
                nc.vector.tensor_copy(out=s_sb[0:1, 0, :], in_=sum_ps[:, :])
                nc.vector.tensor_copy(out=s_sb[0:1, 1, :], in_=sq_ps[:, :])
                mraw = pst.tile([128, QTOK], f32, tag="meanF", name="mraw")
                nc.gpsimd.partition_broadcast(
                    out_ap=mraw[:, :], in_ap=s_sb[:, 0, :], channels=128
                )
                meanb = pst.tile([128, QTOK], bf, tag="mean", name="meanb")
                nc.vector.tensor_scalar_mul(meanb, mraw, 1.0 / D)
                nc.vector.tensor_scalar_mul(mraw, mraw, 1.0 / D)
                varb = pst.tile([128, QTOK], f32, tag="var", name="varb")
                nc.gpsimd.partition_broadcast(
                    out_ap=varb[:, :], in_ap=s_sb[:, 1, :], channels=128
                )
                nc.vector.tensor_scalar_mul(varb, varb, 1.0 / D)
                nc.vector.tensor_mul(mraw, mraw, mraw)
                nc.vector.tensor_sub(varb, varb, mraw)
                nc.vector.tensor_scalar_add(varb, varb, EPS)
                vrc = pst.tile([128, QTOK], f32, tag="vrc", name="vrc")
                nc.vector.reciprocal_approx_fast(out=vrc, in_=varb)
                rstdb = pst.tile([128, QTOK], bf, tag="rstd", name="rstdb")
                nc.scalar.activation(out=rstdb, in_=vrc, func=AF.Sqrt)
                for i in range(8):
                    eng = nc.vector if i < 5 else nc.gpsimd
                    t1 = psq.tile([128, QTOK], bf, tag="sq", name="t1")
                    eng.tensor_sub(t1, x_in[:, i, :], meanb)
                    eng.tensor_mul(t1, t1, rstdb)
                    if out_dma:
                        if i % 2 == 0:
                            o = pout.tile([128, QTOK], f32, tag="o", name="o")
                        else:
                            o = mraw
                        eng.tensor_scalar(
                            out=o, in0=t1,
                            scalar1=g_sb[:, i : i + 1], scalar2=b_sb[:, i : i + 1],
                            op0=Alu.mult, op1=Alu.add,
                        )
                        nc.sync.dma_start(out=outT_r[:, i, :], in_=o)
                    else:
                        eng.tensor_scalar(
                            out=out_tile[:, i, :], in0=t1,
                            scalar1=g_sb[:, i : i + 1], scalar2=b_sb[:, i : i + 1],
                            op0=Alu.mult, op1=Alu.add,
                        )

            def eight_psums():
                a = [ppp.tile([128, QTOK], f32, tag="acc", name=f"fa{_i}")
                     for _i in range(2)]
                b = [ppc.tile([128, QTOK], f32, tag="pc", name=f"fb{_i}")
                     for _i in range(2)]
                c_ = [pps.tile([128, 2, QTOK], f32, tag="ps", name=f"fc{_i}")
                      for _i in range(2)]
                return [a[0][:, :], a[1][:, :], b[0][:, :], b[1][:, :],
                        c_[0][:, 0, :], c_[0][:, 1, :],
                        c_[1][:, 0, :], c_[1][:, 1, :]]

            # ================= self-attention =================
            ca_w = {}
            enc_xb = []

            def prefetch_ca():
                for wn in ("wk", "wv", "wq"):
                    t = pw.tile([128, 8, D], bf, tag="w", name=f"ca_{wn}")
                    nc.sync.dma_start(out=t, in_=w[f"ca_{wn}"][:, :, :])
                    ca_w[wn] = t
                for blk in range(2):
                    t = pxb.tile([128, 8, QTOK], bf, tag="xb", name="exb")
                    nc.gpsimd.dma_start(
                        out=t, in_=encT[:, :, blk * 512 : (blk + 1) * 512]
                    )
                    enc_xb.append(t)

            ffn_w = {}

            def prefetch_ffn():
                for nm, hnd, sl in (
                    ("f1s0", fc1_w, (slice(None), slice(None), slice(0, D))),
                    ("f2s0", fc2_w, (slice(None), slice(0, 8), slice(None))),
                    ("f1s1", fc1_w, (slice(None), slice(None), slice(D, 2 * D))),
                ):
                    t = pw.tile([128, 8, D], bf, tag="w", name=nm)
                    nc.sync.dma_start(out=t, in_=hnd[sl])
                    ffn_w[nm] = t

            ctx1 = pctx.tile([128, 8, QTOK], bf, tag="ctx", name="ctx1")
            sum1, sq1 = attention("sa", xT, xq_sb, ctx1, xq_sb,
                                  prefetch=prefetch_ca)
            x2 = pxa.tile([128, 8, QTOK], bf, tag="xa", name="x2")

            # ================= cross-attention =================
            # LN1 is emitted as `mid` inside attention("ca"): its DVE chain
            # runs while the PE streams CA's K/V projections (enc-based, no
            # dependency on x2), instead of blocking their PSUM drains.
            ctx2 = pctx.tile([128, 8, QTOK], bf, tag="ctx", name="ctx2")
            sum2, sq2 = attention("ca", encT, x2, ctx2, x2,
                                  prefetch=prefetch_ffn, pre_xb=enc_xb,
                                  pre_w=ca_w,
                                  mid=lambda: ln_finish(sum1, sq1, "ln1",
                                                        xq_sb, x2))
            x3 = pqt.tile([128, 8, QTOK], bf, tag="qt", name="x3")
            ln_finish(sum2, sq2, "ln2", x2, x3)

            # ================= feed-forward =================
            h2 = ph2.tile([128, 8, QTOK], bf, tag="h2", name="h2")
            for qtr in range(4):
                if qtr == 0:
                    f1s = ffn_w["f1s0"]
                elif qtr == 1:
                    f1s = ffn_w["f1s1"]
                else:
                    f1s = pw.tile([128, 8, D], bf, tag="w", name="f1s")
                    nc.sync.dma_start(
                        out=f1s, in_=fc1_w[:, :, qtr * D : (qtr + 1) * D]
                    )
                h1a = eight_psums()
                h1q = pctx.tile([128, 8, QTOK], bf, tag="ctx", name="h1q")
                for f in range(8):
                    for c in range(8):
                        nc.tensor.matmul(
                            h1a[f],
                            f1s[:, c, f * 128 : (f + 1) * 128],
                            x3[:, c, :],
                            start=(c == 0),
                            stop=(c == 7),
                        )
                    nc.scalar.activation(
                        out=h1q[:, f, :], in_=h1a[f], func=AF.Relu,
                        bias=fc1b_sb[:, qtr * 8 + f : qtr * 8 + f + 1],
                    )
                if qtr == 0:
                    f2s = ffn_w["f2s0"]
                else:
                    f2s = pw.tile([128, 8, D], bf, tag="w", name="f2s")
                    nc.sync.dma_start(
                        out=f2s, in_=fc2_w[:, qtr * 8 : (qtr + 1) * 8, :]
                    )
                h2a = eight_psums()
                for i in range(8):
                    for f in range(8):
                        nc.tensor.matmul(
                            h2a[i],
                            f2s[:, f, i * 128 : (i + 1) * 128],
                            h1q[:, f, :],
                            start=(f == 0),
                            stop=(f == 7),
                        )
                    if qtr == 0:
                        nc.vector.tensor_copy(out=h2[:, i, :], in_=h2a[i])
                    else:
                        hc = psq.tile([128, QTOK], bf, tag="sq", name="hc")
                        nc.vector.tensor_copy(out=hc, in_=h2a[i])
                        nc.vector.tensor_tensor(
                            out=h2[:, i, :], in0=h2[:, i, :], in1=hc[:, :],
                            op=Alu.add,
                        )

            # residual + LN3 (stats interleaved per tile) + output DMA
            sum3 = ppp.tile([1, QTOK], f32, tag="acc", name="sum3")
            sq3 = ppp.tile([1, QTOK], f32, tag="acc", name="sq3")
            for i in range(8):
                nc.vector.tensor_scalar_add(
                    h2[:, i, :], h2[:, i, :], fc2b_sb[:, i : i + 1]
                )
                nc.vector.tensor_tensor(
                    out=x3[:, i, :], in0=x3[:, i, :], in1=h2[:, i, :],
                    op=Alu.add,
                )
                nc.tensor.matmul(
                    sum3[:, :], ones[:, 0:1], x3[:, i, :],
                    start=(i == 0), stop=(i == 7),
                )
                sqt = psq.tile([128, QTOK], bf, tag="sq", name="sqt3")
                nc.vector.tensor_tensor(
                    out=sqt, in0=x3[:, i, :], in1=x3[:, i, :], op=Alu.mult
                )
                nc.tensor.matmul(
                    sq3[:, :], ones[:, 0:1], sqt[:, :],
                    start=(i == 0), stop=(i == 7),
                )
            ln_finish(sum3, sq3, "ln3", x3, None, out_dma=True)

    nc.compile()
    return nc


def _get_program(mode=None):
    key = "bf16"
    if key not in _PROGRAM_CACHE:
        _PROGRAM_CACHE[key] = _build_program()
    return _PROGRAM_CACHE[key]


def _make_in_maps(inputs):
    import ml_dtypes

    f = np.float32
    bf = ml_dtypes.bfloat16

    def cpn(a, c):  # [c*128, N] -> [128, c, N]
        a = np.asarray(a, dtype=f)
        return np.ascontiguousarray(
            a.reshape(c, 128, a.shape[1]).transpose(1, 0, 2).astype(bf)
        )

    def colmajor8(v):
        return np.ascontiguousarray(np.asarray(v).reshape(8, 128).T.astype(f))

    shared = {}
    for p in ("sa", "ca"):
        for wn in ("wq", "wk", "wv"):
            shared[f"{p}_{wn}"] = cpn(inputs[f"{p}_{wn}"], 8)
        for bn in ("bq", "bk"):
            shared[f"{p}_{bn}"] = colmajor8(inputs[f"{p}_{bn}"])
        shared[f"{p}_bv"] = np.ascontiguousarray(
            np.broadcast_to(np.asarray(inputs[f"{p}_bv"], dtype=f), (128, D))
        ).astype(bf)
    shared["fc1_w"] = cpn(inputs["fc1_w"], 8)
    shared["fc2_w"] = cpn(inputs["fc2_w"], 32)
    shared["fc1_b"] = np.ascontiguousarray(
        np.asarray(inputs["fc1_b"]).reshape(32, 128).T.astype(f)
    )
    shared["fc2_b"] = colmajor8(inputs["fc2_b"])
    for i in (1, 2, 3):
        shared[f"ln{i}_g"] = colmajor8(inputs[f"ln{i}_g"])
        shared[f"ln{i}_b"] = colmajor8(inputs[f"ln{i}_b"])

    hs = np.asarray(inputs["hidden_states"], dtype=f)
    enc = np.asarray(inputs["encoder_hidden_states"], dtype=f)
    in_maps = []
    xt_c = {}
    enc_c = {}
    for b in range(2):
        xt_c[b] = cpn(np.ascontiguousarray(hs[b].T), 8)
        enc_c[b] = cpn(np.ascontiguousarray(enc[b].T), 8)
    for c in range(NCORES):
        b, q0 = c // 4, (c % 4) * QTOK
        m = dict(shared)
        m["xT"] = xt_c[b]
        m["xqT"] = np.ascontiguousarray(xt_c[b][:, :, q0 : q0 + QTOK])
        m["encT"] = enc_c[b]
        in_maps.append(m)
    return in_maps


def kernel(**inputs):
    from concourse.bass_utils import run_bass_kernel_spmd

    nc = _get_program()
    in_maps = _make_in_maps(inputs)
    res = run_bass_kernel_spmd(nc, in_maps, core_ids=list(range(NCORES)))
    out = np.empty((2, S, D), np.float32)
    for c in range(NCORES):
        b, q0 = c // 4, (c % 4) * QTOK
        out[b, q0 : q0 + QTOK, :] = res.results[c]["outT"].T
    return out



# revision 6
# speedup vs baseline: 1.0061x; 1.0061x over previous
"""Trainium2 Bass kernel for nn_DecoderLayer (self-attn + cross-attn + FFN).

Sharding: 8 cores = 2 batches x 4 query-blocks of 512 tokens (data/sequence
parallel, zero collectives). Each core recomputes the full K/V for its batch
and runs its 512 queries through the whole layer.

v2: all matmul operands bf16 (weights/activations cast host-side where
possible) -- same 1 cyc/row PE rate as f32r but half the DMA bytes and half
the SBUF footprint. K/V for all 4 head-groups stay resident so X/enc stream
exactly once per attention. Per-attention order is K/V proj -> Q proj ->
heads, so cross-attention K/V (which needs only encT) overlaps LN1's vector
chain. Two heads sharing a kt dtile are interleaved per j-chunk so the PE
stays fed through the scalar-engine exp latency; residual adds + LayerNorm
statistics are deferred to one block after the head loop so the stat
matmuls never stall the tensor stream on a softmax-normalize chain.
Next-phase weights (CA, FFN slabs) and encoder blocks are prefetched
before each head loop.
"""

import sys

if "/opt/trn_rl_repo" not in sys.path:
    sys.path.insert(0, "/opt/trn_rl_repo")

import numpy as np

D = 1024
S = 2048
QTOK = 512          # queries per core
H = 16
HD = 64
FFN = 4096
EPS = 1e-5
NCORES = 8

_PROGRAM_CACHE = {}


def _build_program():
    import contextlib

    import concourse.bacc as bacc
    import concourse.tile as tile
    from concourse import mybir

    f32 = mybir.dt.float32
    bf = mybir.dt.bfloat16
    AF = mybir.ActivationFunctionType
    Alu = mybir.AluOpType

    nc = bacc.Bacc("TRN2", target_bir_lowering=False)

    def din(name, shape, dt=bf):
        return nc.declare_dram_parameter(name, list(shape), dt, isOutput=False)

    xT = din("xT", (128, 8, S))
    xqT = din("xqT", (128, 8, QTOK))
    encT = din("encT", (128, 8, S))
    w = {}
    for p in ("sa", "ca"):
        for wn in ("wq", "wk", "wv"):
            w[f"{p}_{wn}"] = din(f"{p}_{wn}", (128, 8, D))
        for bn in ("bq", "bk"):
            w[f"{p}_{bn}"] = din(f"{p}_{bn}", (128, 8), f32)
        w[f"{p}_bv"] = din(f"{p}_bv", (128, D))
    fc1_w = din("fc1_w", (128, 8, FFN))
    fc2_w = din("fc2_w", (128, 32, D))
    fc1_b = din("fc1_b", (128, 32), f32)
    fc2_b = din("fc2_b", (128, 8), f32)
    for i in (1, 2, 3):
        w[f"ln{i}_g"] = din(f"ln{i}_g", (128, 8), f32)
        w[f"ln{i}_b"] = din(f"ln{i}_b", (128, 8), f32)
    outT = nc.declare_dram_parameter("outT", [D, QTOK], f32, isOutput=True)
    outT_r = outT.rearrange("(i p) t -> p i t", p=128)

    with tile.TileContext(nc) as tc:
        with contextlib.ExitStack() as ctx:
            consts = ctx.enter_context(tc.tile_pool(name="consts", bufs=1))
            pw = ctx.enter_context(tc.tile_pool(name="pw", bufs=3))
            pkt = ctx.enter_context(tc.tile_pool(name="pkt", bufs=8))
            pvp = ctx.enter_context(tc.tile_pool(name="pvp", bufs=1))
            pqt = ctx.enter_context(tc.tile_pool(name="pqt", bufs=1))
            pxb = ctx.enter_context(tc.tile_pool(name="pxb", bufs=2))
            pctx = ctx.enter_context(tc.tile_pool(name="pctx", bufs=1))
            pxa = ctx.enter_context(tc.tile_pool(name="pxa", bufs=1))
            ph2 = ctx.enter_context(tc.tile_pool(name="ph2", bufs=1))
            pst = ctx.enter_context(tc.tile_pool(name="pst", bufs=1))
            psq = ctx.enter_context(tc.tile_pool(name="psq", bufs=2))
            pexp = ctx.enter_context(tc.tile_pool(name="pexp", bufs=4))
            pout = ctx.enter_context(tc.tile_pool(name="pout", bufs=1))
            ppp = ctx.enter_context(tc.tile_pool(name="ppp", bufs=2, space="PSUM"))
            pps = ctx.enter_context(tc.tile_pool(name="pps", bufs=2, space="PSUM"))
            ppc = ctx.enter_context(tc.tile_pool(name="ppc", bufs=2, space="PSUM"))

            # ---- constants ----
            ones = consts.tile([128, 1], bf, tag="ones")
            nc.vector.memset(ones, 1.0)
            xq_sb = consts.tile([128, 8, QTOK], bf, tag="xq")
            nc.gpsimd.dma_start(out=xq_sb, in_=xqT[:, :, :])
            sb = {}
            for name, hnd in w.items():
                if name.endswith(("wq", "wk", "wv")):
                    continue
                dt_ = bf if name.endswith("_bv") else f32
                sb[name] = consts.tile(
                    [128, hnd.shape[1]], dt_, tag=name, name=name
                )
                nc.scalar.dma_start(out=sb[name], in_=hnd[:, :])
            fc1b_sb = consts.tile([128, 32], f32, tag="fc1b")
            nc.scalar.dma_start(out=fc1b_sb, in_=fc1_b[:, :])
            fc2b_sb = consts.tile([128, 8], f32, tag="fc2b")
            nc.scalar.dma_start(out=fc2b_sb, in_=fc2_b[:, :])

            def attention(pfx, src_dram, q_sb, ctx_tile, resid,
                          prefetch=None, pre_xb=None, pre_w=None, mid=None):
                """One MHA. K/V proj (streaming src once) -> Q proj -> heads.
                After each head pair, the residual add + LN stat matmuls for
                that dtile are emitted (accumulating into sum_ps/sq_ps, which
                are allocated last on the 'acc' ring so nothing displaces
                them mid-accumulation). Returns (sum_ps, sq_ps)."""
                bq_sb, bk_sb, bv_sb = sb[f"{pfx}_bq"], sb[f"{pfx}_bk"], sb[f"{pfx}_bv"]
                if pre_w is not None:
                    wk, wv, wq = pre_w["wk"], pre_w["wv"], pre_w["wq"]
                else:
                    wk = pw.tile([128, 8, D], bf, tag="w", name="wk")
                    nc.sync.dma_start(out=wk, in_=w[f"{pfx}_wk"][:, :, :])
                    wv = pw.tile([128, 8, D], bf, tag="w", name="wv")
                    nc.sync.dma_start(out=wv, in_=w[f"{pfx}_wv"][:, :, :])
                    wq = pw.tile([128, 8, D], bf, tag="w", name="wq")
                    nc.sync.dma_start(out=wq, in_=w[f"{pfx}_wq"][:, :, :])

                kts = [
                    pkt.tile([128, S], bf, tag="kt", name=f"kt{_d}")
                    for _d in range(8)
                ]
                vp = pvp.tile([128, 16, 4, 261], bf, tag="vp", name="vp")
                # ones at col h*65 of each group block, plus trailing col 260
                nc.vector.memset(
                    vp[:, :, :, 0 : 65 * 4].rearrange(
                        "p j g (h u) -> p j g h u", u=65
                    )[:, :, :, :, 0:1],
                    1.0,
                )
                nc.vector.memset(vp[:, :, :, 260:261], 1.0)

                # ---- K/V projections, one pass over src ----
                for blk in range(4):
                    if pre_xb is not None and blk < len(pre_xb):
                        xb = pre_xb[blk]
                    else:
                        xb = pxb.tile([128, 8, QTOK], bf, tag="xb", name="xb")
                        nc.gpsimd.dma_start(
                            out=xb,
                            in_=src_dram[:, :, blk * 512 : (blk + 1) * 512],
                        )
                    for dt in range(8):
                        kacc = pps.tile([128, QTOK], f32, tag="ps", name="kacc")
                        for c in range(8):
                            nc.tensor.matmul(
                                kacc[:, :],
                                wk[:, c, dt * 128 : (dt + 1) * 128],
                                xb[:, c, :],
                                start=(c == 0),
                                stop=(c == 7),
                            )
                        nc.vector.tensor_scalar_add(
                            kts[dt][:, blk * 512 : (blk + 1) * 512],
                            kacc[:, :],
                            bk_sb[:, dt : dt + 1],
                        )
                    for tt in range(4):
                        j = blk * 4 + tt
                        for hf in range(2):
                            vacc = pps.tile([128, QTOK], f32, tag="ps",
                                            name="vacc")
                            for c in range(8):
                                nc.tensor.matmul(
                                    vacc[:, :],
                                    xb[:, c, tt * 128 : (tt + 1) * 128],
                                    wv[:, c, hf * 512 : (hf + 1) * 512],
                                    start=(c == 0),
                                    stop=(c == 7),
                                )
                            vc = psq.tile([128, QTOK], bf, tag="sq",
                                          name="vc")
                            nc.vector.tensor_copy(out=vc, in_=vacc[:, :])
                            for gg in range(2):
                                g = hf * 2 + gg
                                dst = vp[:, j, g, 1:261].rearrange(
                                    "p (h u) -> p h u", u=65
                                )[:, :, 0:64]
                                nc.vector.tensor_tensor(
                                    out=dst,
                                    in0=vc[:, gg * 256 : (gg + 1) * 256]
                                    .rearrange("p (h u) -> p h u", u=64),
                                    in1=bv_sb[:, g * 256 : (g + 1) * 256]
                                    .rearrange("p (h u) -> p h u", u=64),
                                    op=Alu.add,
                                )

                if mid is not None:
                    mid()
                # ---- Q projection ----
                qt = pqt.tile([128, 8, QTOK], bf, tag="qt", name="qt")
                for dt in range(8):
                    qacc = pps.tile([128, QTOK], f32, tag="ps", name="qacc")
                    for c in range(8):
                        nc.tensor.matmul(
                            qacc[:, :],
                            wq[:, c, dt * 128 : (dt + 1) * 128],
                            q_sb[:, c, :],
                            start=(c == 0),
                            stop=(c == 7),
                        )
                    nc.vector.tensor_scalar_add(
                        qt[:, dt, :], qacc[:, :], bq_sb[:, dt : dt + 1]
                    )

                if prefetch is not None:
                    prefetch()
                # ---- heads (+ interleaved LN statistic accumulation) ----
                # two heads (sharing a kt dtile) are interleaved per j-chunk
                # so the PE always has ~8 matmuls queued per exp window and
                # never idles into a lower p-state waiting on the scalar exp.
                sum_ps = ppp.tile([1, QTOK], f32, tag="acc", name="sum_ps")
                sq_ps = ppp.tile([1, QTOK], f32, tag="acc", name="sq_ps")
                for hp in range(8):
                    dt = hp
                    g = dt // 2
                    pcs = []
                    ets = [[], []]
                    for par in range(2):
                        pcs.append(ppc.tile([128, QTOK], f32, tag="pc",
                                            name=f"pc{par}"))
                    for jg in range(9):
                        if jg < 8:
                            for par in range(2):
                                ps = pps.tile([128, 2, QTOK], f32, tag="ps",
                                              name="ps")
                                for js in range(2):
                                    j = jg * 2 + js
                                    nc.tensor.matmul(
                                        ps[:, js, :],
                                        kts[dt][par * 64 : par * 64 + 64,
                                                j * 128 : (j + 1) * 128],
                                        qt[par * 64 : par * 64 + 64, dt, :],
                                        start=True,
                                        stop=True,
                                    )
                                et = pexp.tile([128, 2, QTOK], bf, tag="et",
                                               name="et")
                                nc.scalar.activation(
                                    out=et, in_=ps, func=AF.Exp,
                                    scale=1.0 / (HD ** 0.5),
                                )
                                ets[par].append(et)
                        if jg >= 1:
                            jj = jg - 1
                            for par in range(2):
                                h = 2 * hp + par
                                vlo = (h % 4) * 65 + 1
                                for js in range(2):
                                    j = jj * 2 + js
                                    nc.tensor.matmul(
                                        pcs[par][0:65, :],
                                        vp[:, j, g, vlo : vlo + 65],
                                        ets[par][jj][:, js, :],
                                        start=(jj == 0 and js == 0),
                                        stop=(jj == 7 and js == 1),
                                    )
                    # normalize both heads: reciprocal of denom (psum row
                    # 64), broadcast, multiply (DVE has no divide op)
                    for par in range(2):
                        pc = pcs[par]
                        dn = psq.tile([128, QTOK], f32, tag="sq", name="dn")
                        nc.vector.tensor_copy(out=dn[64:65, :],
                                              in_=pc[64:65, :])
                        nc.gpsimd.dma_start(out=dn[0:1, :], in_=dn[64:65, :])
                        dn2 = psq.tile([128, QTOK], f32, tag="sq",
                                       name="dn2")
                        nc.vector.reciprocal_approx_fast(out=dn2[0:1, :],
                                                         in_=dn[0:1, :])
                        db = pexp.tile([128, QTOK], f32, tag="db", name="db",
                                       bufs=2)
                        nc.gpsimd.partition_broadcast(
                            out_ap=db[0:64, :], in_ap=dn2[0:64, :], channels=64
                        )
                        if par == 0:
                            nc.vector.tensor_tensor(
                                out=ctx_tile[0:64, dt, :],
                                in0=pc[0:64, :],
                                in1=db[0:64, :],
                                op=Alu.mult,
                            )
                        else:
                            cn = psq.tile([128, QTOK], bf, tag="sq",
                                          name="cn")
                            nc.vector.tensor_tensor(
                                out=cn[0:64, :],
                                in0=pc[0:64, :],
                                in1=db[0:64, :],
                                op=Alu.mult,
                            )
                            nc.gpsimd.dma_start(
                                out=ctx_tile[64:128, dt, :], in_=cn[0:64, :]
                            )
                # residual adds + LN stats, deferred out of the head loop
                # so the stat matmuls never block the tensor stream on a
                # normalize chain mid-heads; only the last pair's chain is
                # exposed, once.
                for i in range(8):
                    nc.vector.tensor_tensor(
                        out=resid[:, i, :], in0=resid[:, i, :],
                        in1=ctx_tile[:, i, :], op=Alu.add,
                    )
                    nc.tensor.matmul(
                        sum_ps[:, :], ones[:, 0:1], resid[:, i, :],
                        start=(i == 0), stop=(i == 7),
                    )
                    sqt = psq.tile([128, QTOK], bf, tag="sq", name="sqt")
                    nc.vector.tensor_tensor(
                        out=sqt, in0=resid[:, i, :], in1=resid[:, i, :],
                        op=Alu.mult,
                    )
                    nc.tensor.matmul(
                        sq_ps[:, :], ones[:, 0:1], sqt[:, :],
                        start=(i == 0), stop=(i == 7),
                    )
                return sum_ps, sq_ps

            def ln_finish(sum_ps, sq_ps, gname, x_in, out_tile, out_dma=False):
                """Broadcast stats, then out = LN(x_in) * g + b per tile."""
                g_sb, b_sb = sb[f"{gname}_g"], sb[f"{gname}_b"]
                s_sb = pst.tile([128, 2, QTOK], f32, tag="sb", name="s_sb")
                nc.vector.tensor_copy(out=s_sb[0:1, 0, :], in_=sum_ps[:, :])
                nc.vector.tensor_copy(out=s_sb[0:1, 1, :], in_=sq_ps[:, :])
                mraw = pst.tile([128, QTOK], f32, tag="meanF", name="mraw")
                nc.gpsimd.partition_broadcast(
                    out_ap=mraw[:, :], in_ap=s_sb[:, 0, :], channels=128
                )
                meanb = pst.tile([128, QTOK], bf, tag="mean", name="meanb")
                nc.vector.tensor_scalar_mul(meanb, mraw, 1.0 / D)
                nc.vector.tensor_scalar_mul(mraw, mraw, 1.0 / D)
                varb = pst.tile([128, QTOK], f32, tag="var", name="varb")
                nc.gpsimd.partition_broadcast(
                    out_ap=varb[:, :], in_ap=s_sb[:, 1, :], channels=128
                )
                nc.vector.tensor_scalar_mul(varb, varb, 1.0 / D)
                nc.vector.tensor_mul(mraw, mraw, mraw)
                nc.vector.tensor_sub(varb, varb, mraw)
                nc.vector.tensor_scalar_add(varb, varb, EPS)
                vrc = pst.tile([128, QTOK], f32, tag="vrc", name="vrc")
                nc.vector.reciprocal_approx_fast(out=vrc, in_=varb)
                rstdb = pst.tile([128, QTOK], bf, tag="rstd", name="rstdb")
                nc.scalar.activation(out=rstdb, in_=vrc, func=AF.Sqrt)
                for i in range(8):
                    eng = nc.vector if i < 5 else nc.gpsimd
                    t1 = psq.tile([128, QTOK], bf, tag="sq", name="t1")
                    eng.tensor_sub(t1, x_in[:, i, :], meanb)
                    eng.tensor_mul(t1, t1, rstdb)
                    if out_dma:
                        if i % 2 == 0:
                            o = pout.tile([128, QTOK], f32, tag="o", name="o")
                        else:
                            o = mraw
                        eng.tensor_scalar(
                            out=o, in0=t1,
                            scalar1=g_sb[:, i : i + 1], scalar2=b_sb[:, i : i + 1],
                            op0=Alu.mult, op1=Alu.add,
                        )
                        nc.sync.dma_start(out=outT_r[:, i, :], in_=o)
                    else:
                        eng.tensor_scalar(
                            out=out_tile[:, i, :], in0=t1,
                            scalar1=g_sb[:, i : i + 1], scalar2=b_sb[:, i : i + 1],
                            op0=Alu.mult, op1=Alu.add,
                        )

            def eight_psums():
                a = [ppp.tile([128, QTOK], f32, tag="acc", name=f"fa{_i}")
                     for _i in range(2)]
                b = [ppc.tile([128, QTOK], f32, tag="pc", name=f"fb{_i}")
                     for _i in range(2)]
                c_ = [pps.tile([128, 2, QTOK], f32, tag="ps", name=f"fc{_i}")
                      for _i in range(2)]
                return [a[0][:, :], a[1][:, :], b[0][:, :], b[1][:, :],
                        c_[0][:, 0, :], c_[0][:, 1, :],
                        c_[1][:, 0, :], c_[1][:, 1, :]]

            # ================= self-attention =================
            ca_w = {}
            enc_xb = []

            def prefetch_ca():
                for wn in ("wk", "wv", "wq"):
                    t = pw.tile([128, 8, D], bf, tag="w", name=f"ca_{wn}")
                    nc.sync.dma_start(out=t, in_=w[f"ca_{wn}"][:, :, :])
                    ca_w[wn] = t
                for blk in range(2):
                    t = pxb.tile([128, 8, QTOK], bf, tag="xb", name="exb")
                    nc.gpsimd.dma_start(
                        out=t, in_=encT[:, :, blk * 512 : (blk + 1) * 512]
                    )
                    enc_xb.append(t)

            ffn_w = {}

            def prefetch_ffn():
                for nm, hnd, sl in (
                    ("f1s0", fc1_w, (slice(None), slice(None), slice(0, D))),
                    ("f2s0", fc2_w, (slice(None), slice(0, 8), slice(None))),
                    ("f1s1", fc1_w, (slice(None), slice(None), slice(D, 2 * D))),
                ):
                    t = pw.tile([128, 8, D], bf, tag="w", name=nm)
                    nc.sync.dma_start(out=t, in_=hnd[sl])
                    ffn_w[nm] = t

            ctx1 = pctx.tile([128, 8, QTOK], bf, tag="ctx", name="ctx1")
            sum1, sq1 = attention("sa", xT, xq_sb, ctx1, xq_sb,
                                  prefetch=prefetch_ca)
            x2 = pxa.tile([128, 8, QTOK], bf, tag="xa", name="x2")

            # ================= cross-attention =================
            # LN1 is emitted as `mid` inside attention("ca"): its DVE chain
            # runs while the PE streams CA's K/V projections (enc-based, no
            # dependency on x2), instead of blocking their PSUM drains.
            ctx2 = pctx.tile([128, 8, QTOK], bf, tag="ctx", name="ctx2")
            sum2, sq2 = attention("ca", encT, x2, ctx2, x2,
                                  prefetch=prefetch_ffn, pre_xb=enc_xb,
                                  pre_w=ca_w,
                                  mid=lambda: ln_finish(sum1, sq1, "ln1",
                                                        xq_sb, x2))
            x3 = pqt.tile([128, 8, QTOK], bf, tag="qt", name="x3")
            ln_finish(sum2, sq2, "ln2", x2, x3)

            # ================= feed-forward =================
            h2 = ph2.tile([128, 8, QTOK], bf, tag="h2", name="h2")
            for qtr in range(4):
                if qtr == 0:
                    f1s = ffn_w["f1s0"]
                elif qtr == 1:
                    f1s = ffn_w["f1s1"]
                else:
                    f1s = pw.tile([128, 8, D], bf, tag="w", name="f1s")
                    nc.sync.dma_start(
                        out=f1s, in_=fc1_w[:, :, qtr * D : (qtr + 1) * D]
                    )
                h1a = eight_psums()
                h1q = pctx.tile([128, 8, QTOK], bf, tag="ctx", name="h1q")
                for f in range(8):
                    for c in range(8):
                        nc.tensor.matmul(
                            h1a[f],
                            f1s[:, c, f * 128 : (f + 1) * 128],
                            x3[:, c, :],
                            start=(c == 0),
                            stop=(c == 7),
                        )
                    nc.scalar.activation(
                        out=h1q[:, f, :], in_=h1a[f], func=AF.Relu,
                        bias=fc1b_sb[:, qtr * 8 + f : qtr * 8 + f + 1],
                    )
                if qtr == 0:
                    f2s = ffn_w["f2s0"]
                else:
                    f2s = pw.tile([128, 8, D], bf, tag="w", name="f2s")
                    nc.sync.dma_start(
                        out=f2s, in_=fc2_w[:, qtr * 8 : (qtr + 1) * 8, :]
                    )
                h2a = eight_psums()
                for i in range(8):
                    for f in range(8):
                        nc.tensor.matmul(
                            h2a[i],
                            f2s[:, f, i * 128 : (i + 1) * 128],
                            h1q[:, f, :],
                            start=(f == 0),
                            stop=(f == 7),
                        )
                    if qtr == 0:
                        nc.vector.tensor_copy(out=h2[:, i, :], in_=h2a[i])
                    else:
                        hc = psq.tile([128, QTOK], bf, tag="sq", name="hc")
                        nc.vector.tensor_copy(out=hc, in_=h2a[i])
                        nc.vector.tensor_tensor(
                            out=h2[:, i, :], in0=h2[:, i, :], in1=hc[:, :],
                            op=Alu.add,
                        )

            # residual + LN3 (stats interleaved per tile) + output DMA
            sum3 = ppp.tile([1, QTOK], f32, tag="acc", name="sum3")
            sq3 = ppp.tile([1, QTOK], f32, tag="acc", name="sq3")
            for i in range(8):
                nc.vector.tensor_scalar_add(
                    h2[:, i, :], h2[:, i, :], fc2b_sb[:, i : i + 1]
                )
                nc.vector.tensor_tensor(
                    out=x3[:, i, :], in0=x3[:, i, :], in1=h2[:, i, :],
                    op=Alu.add,
                )
                nc.tensor.matmul(
                    sum3[:, :], ones[:, 0:1], x3[:, i, :],
                    start=(i == 0), stop=(i == 7),
                )
                sqt = psq.tile([128, QTOK], bf, tag="sq", name="sqt3")
                nc.vector.tensor_tensor(
                    out=sqt, in0=x3[:, i, :], in1=x3[:, i, :], op=Alu.mult
                )
                nc.tensor.matmul(
                    sq3[:, :], ones[:, 0:1], sqt[:, :],
                    start=(i == 0), stop=(i == 7),
                )
            ln_finish(sum3, sq3, "ln3", x3, None, out_dma=True)

    nc.compile()
    return nc


def _get_program(mode=None):
    key = "bf16"
    if key not in _PROGRAM_CACHE:
        _PROGRAM_CACHE[key] = _build_program()
    return _PROGRAM_CACHE[key]


def _make_in_maps(inputs):
    import ml_dtypes

    f = np.float32
    bf = ml_dtypes.bfloat16

    def cpn(a, c):  # [c*128, N] -> [128, c, N]
        a = np.asarray(a, dtype=f)
        return np.ascontiguousarray(
            a.reshape(c, 128, a.shape[1]).transpose(1, 0, 2).astype(bf)
        )

    def colmajor8(v):
        return np.ascontiguousarray(np.asarray(v).reshape(8, 128).T.astype(f))

    shared = {}
    for p in ("sa", "ca"):
        for wn in ("wq", "wk", "wv"):
            shared[f"{p}_{wn}"] = cpn(inputs[f"{p}_{wn}"], 8)
        for bn in ("bq", "bk"):
            shared[f"{p}_{bn}"] = colmajor8(inputs[f"{p}_{bn}"])
        shared[f"{p}_bv"] = np.ascontiguousarray(
            np.broadcast_to(np.asarray(inputs[f"{p}_bv"], dtype=f), (128, D))
        ).astype(bf)
    shared["fc1_w"] = cpn(inputs["fc1_w"], 8)
    shared["fc2_w"] = cpn(inputs["fc2_w"], 32)
    shared["fc1_b"] = np.ascontiguousarray(
        np.asarray(inputs["fc1_b"]).reshape(32, 128).T.astype(f)
    )
    shared["fc2_b"] = colmajor8(inputs["fc2_b"])
    for i in (1, 2, 3):
        shared[f"ln{i}_g"] = colmajor8(inputs[f"ln{i}_g"])
        shared[f"ln{i}_b"] = colmajor8(inputs[f"ln{i}_b"])

    hs = np.asarray(inputs["hidden_states"], dtype=f)
    enc = np.asarray(inputs["encoder_hidden_states"], dtype=f)
    in_maps = []
    xt_c = {}
    enc_c = {}
    for b in range(2):
        xt_c[b] = cpn(np.ascontiguousarray(hs[b].T), 8)
        enc_c[b] = cpn(np.ascontiguousarray(enc[b].T), 8)
    for c in range(NCORES):
        b, q0 = c // 4, (c % 4) * QTOK
        m = dict(shared)
        m["xT"] = xt_c[b]
        m["xqT"] = np.ascontiguousarray(xt_c[b][:, :, q0 : q0 + QTOK])
        m["encT"] = enc_c[b]
        in_maps.append(m)
    return in_maps


def kernel(**inputs):
    from concourse.bass_utils import run_bass_kernel_spmd

    nc = _get_program()
    in_maps = _make_in_maps(inputs)
    res = run_bass_kernel_spmd(nc, in_maps, core_ids=list(range(NCORES)))
    out = np.empty((2, S, D), np.float32)
    for c in range(NCORES):
        b, q0 = c // 4, (c % 4) * QTOK
        out[b, q0 : q0 + QTOK, :] = res.results[c]["outT"].T
    return out



# revision 8
# speedup vs baseline: 1.1783x; 1.1711x over previous
"""Trainium2 Bass kernel for nn_DecoderLayer (self-attn + cross-attn + FFN).

Sharding: 8 cores = 2 batches x 4 query-blocks of 512 tokens (data/sequence
parallel, zero collectives). Each core recomputes the full K/V for its batch
and runs its 512 queries through the whole layer.

v2: all matmul operands bf16 (weights/activations cast host-side where
possible) -- same 1 cyc/row PE rate as f32r but half the DMA bytes and half
the SBUF footprint. K/V for all 4 head-groups stay resident so X/enc stream
exactly once per attention. Per-attention order is K/V proj -> Q proj ->
heads, so cross-attention K/V (which needs only encT) overlaps LN1's vector
chain. Two heads sharing a kt dtile are interleaved per j-chunk so the PE
stays fed through the scalar-engine exp latency; residual adds + LayerNorm
statistics are deferred to one block after the head loop so the stat
matmuls never stall the tensor stream on a softmax-normalize chain.
Next-phase weights (CA, FFN slabs) and encoder blocks are prefetched
before each head loop.
"""

import sys

if "/opt/trn_rl_repo" not in sys.path:
    sys.path.insert(0, "/opt/trn_rl_repo")

import numpy as np

D = 1024
S = 2048
QTOK = 512          # queries per core
H = 16
HD = 64
FFN = 4096
EPS = 1e-5
NCORES = 8

_PROGRAM_CACHE = {}


def _build_program():
    import contextlib

    import concourse.bacc as bacc
    import concourse.tile as tile
    from concourse import mybir

    f32 = mybir.dt.float32
    bf = mybir.dt.bfloat16
    AF = mybir.ActivationFunctionType
    Alu = mybir.AluOpType

    nc = bacc.Bacc("TRN2", target_bir_lowering=False)

    def din(name, shape, dt=bf):
        return nc.declare_dram_parameter(name, list(shape), dt, isOutput=False)

    xT = din("xT", (128, 8, S))
    xqT = din("xqT", (128, 8, QTOK))
    encT = din("encT", (128, 8, S))
    w = {}
    for p in ("sa", "ca"):
        for wn in ("wq", "wk", "wv"):
            w[f"{p}_{wn}"] = din(f"{p}_{wn}", (128, 8, D))
        for bn in ("bq", "bk"):
            w[f"{p}_{bn}"] = din(f"{p}_{bn}", (128, 8), f32)
        w[f"{p}_bv"] = din(f"{p}_bv", (128, D))
    fc1_w = din("fc1_w", (128, 8, FFN))
    fc2_w = din("fc2_w", (128, 32, D))
    fc1_b = din("fc1_b", (128, 32), f32)
    fc2_b = din("fc2_b", (128, 8), f32)
    for i in (1, 2, 3):
        w[f"ln{i}_g"] = din(f"ln{i}_g", (128, 8), f32)
        w[f"ln{i}_b"] = din(f"ln{i}_b", (128, 8), f32)
    outT = nc.declare_dram_parameter("outT", [D, QTOK], f32, isOutput=True)
    outT_r = outT.rearrange("(i p) t -> p i t", p=128)

    with tile.TileContext(nc) as tc:
        with contextlib.ExitStack() as ctx:
            consts = ctx.enter_context(tc.tile_pool(name="consts", bufs=1))
            pw = ctx.enter_context(tc.tile_pool(name="pw", bufs=3))
            pkt = ctx.enter_context(tc.tile_pool(name="pkt", bufs=8))
            pvp = ctx.enter_context(tc.tile_pool(name="pvp", bufs=1))
            pqt = ctx.enter_context(tc.tile_pool(name="pqt", bufs=1))
            pxb = ctx.enter_context(tc.tile_pool(name="pxb", bufs=2))
            pctx = ctx.enter_context(tc.tile_pool(name="pctx", bufs=1))
            pxa = ctx.enter_context(tc.tile_pool(name="pxa", bufs=1))
            ph2 = ctx.enter_context(tc.tile_pool(name="ph2", bufs=1))
            pst = ctx.enter_context(tc.tile_pool(name="pst", bufs=1))
            psq = ctx.enter_context(tc.tile_pool(name="psq", bufs=2))
            pexp = ctx.enter_context(tc.tile_pool(name="pexp", bufs=4))
            pout = ctx.enter_context(tc.tile_pool(name="pout", bufs=1))
            ppp = ctx.enter_context(tc.tile_pool(name="ppp", bufs=1, space="PSUM"))
            pps = ctx.enter_context(tc.tile_pool(name="pps", bufs=2, space="PSUM"))
            ppc = ctx.enter_context(tc.tile_pool(name="ppc", bufs=3, space="PSUM"))

            # ---- constants ----
            ones = consts.tile([128, 1], bf, tag="ones")
            nc.vector.memset(ones, 1.0)
            xq_sb = consts.tile([128, 8, QTOK], bf, tag="xq")
            nc.gpsimd.dma_start(out=xq_sb, in_=xqT[:, :, :])
            sb = {}
            for name, hnd in w.items():
                if name.endswith(("wq", "wk", "wv")):
                    continue
                dt_ = bf if name.endswith("_bv") else f32
                sb[name] = consts.tile(
                    [128, hnd.shape[1]], dt_, tag=name, name=name
                )
                nc.scalar.dma_start(out=sb[name], in_=hnd[:, :])
            fc1b_sb = consts.tile([128, 32], f32, tag="fc1b")
            nc.scalar.dma_start(out=fc1b_sb, in_=fc1_b[:, :])
            fc2b_sb = consts.tile([128, 8], f32, tag="fc2b")
            nc.scalar.dma_start(out=fc2b_sb, in_=fc2_b[:, :])

            def attention(pfx, src_dram, q_sb, ctx_tile, resid,
                          prefetch=None, pre_xb=None, pre_w=None, mid=None):
                """One MHA. K/V proj (streaming src once) -> Q proj -> heads.
                After each head pair, the residual add + LN stat matmuls for
                that dtile are emitted (accumulating into sum_ps/sq_ps, which
                are allocated last on the 'acc' ring so nothing displaces
                them mid-accumulation). Returns (sum_ps, sq_ps)."""
                bq_sb, bk_sb, bv_sb = sb[f"{pfx}_bq"], sb[f"{pfx}_bk"], sb[f"{pfx}_bv"]
                if pre_w is not None:
                    wk, wv, wq = pre_w["wk"], pre_w["wv"], pre_w["wq"]
                else:
                    wk = pw.tile([128, 8, D], bf, tag="w", name="wk")
                    nc.sync.dma_start(out=wk, in_=w[f"{pfx}_wk"][:, :, :])
                    wv = pw.tile([128, 8, D], bf, tag="w", name="wv")
                    nc.sync.dma_start(out=wv, in_=w[f"{pfx}_wv"][:, :, :])
                    wq = pw.tile([128, 8, D], bf, tag="w", name="wq")
                    nc.sync.dma_start(out=wq, in_=w[f"{pfx}_wq"][:, :, :])

                kts = [
                    pkt.tile([128, S], bf, tag="kt", name=f"kt{_d}")
                    for _d in range(8)
                ]
                vp = pvp.tile([128, 16, 4, 261], bf, tag="vp", name="vp")
                # ones at col h*65 of each group block, plus trailing col 260
                nc.vector.memset(
                    vp[:, :, :, 0 : 65 * 4].rearrange(
                        "p j g (h u) -> p j g h u", u=65
                    )[:, :, :, :, 0:1],
                    1.0,
                )
                nc.vector.memset(vp[:, :, :, 260:261], 1.0)

                # ---- K/V projections, one pass over src ----
                for blk in range(4):
                    if pre_xb is not None and blk < len(pre_xb):
                        xb = pre_xb[blk]
                    else:
                        xb = pxb.tile([128, 8, QTOK], bf, tag="xb", name="xb")
                        nc.gpsimd.dma_start(
                            out=xb,
                            in_=src_dram[:, :, blk * 512 : (blk + 1) * 512],
                        )
                    for dt in range(8):
                        kacc = pps.tile([128, QTOK], f32, tag="ps", name="kacc")
                        for c in range(8):
                            nc.tensor.matmul(
                                kacc[:, :],
                                wk[:, c, dt * 128 : (dt + 1) * 128],
                                xb[:, c, :],
                                start=(c == 0),
                                stop=(c == 7),
                            )
                        nc.vector.tensor_scalar_add(
                            kts[dt][:, blk * 512 : (blk + 1) * 512],
                            kacc[:, :],
                            bk_sb[:, dt : dt + 1],
                        )
                    for tt in range(4):
                        j = blk * 4 + tt
                        for hf in range(2):
                            vacc = pps.tile([128, QTOK], f32, tag="ps",
                                            name="vacc")
                            for c in range(8):
                                nc.tensor.matmul(
                                    vacc[:, :],
                                    xb[:, c, tt * 128 : (tt + 1) * 128],
                                    wv[:, c, hf * 512 : (hf + 1) * 512],
                                    start=(c == 0),
                                    stop=(c == 7),
                                )
                            vc = psq.tile([128, QTOK], bf, tag="sq",
                                          name="vc")
                            nc.vector.tensor_copy(out=vc, in_=vacc[:, :])
                            for gg in range(2):
                                g = hf * 2 + gg
                                dst = vp[:, j, g, 1:261].rearrange(
                                    "p (h u) -> p h u", u=65
                                )[:, :, 0:64]
                                nc.vector.tensor_tensor(
                                    out=dst,
                                    in0=vc[:, gg * 256 : (gg + 1) * 256]
                                    .rearrange("p (h u) -> p h u", u=64),
                                    in1=bv_sb[:, g * 256 : (g + 1) * 256]
                                    .rearrange("p (h u) -> p h u", u=64),
                                    op=Alu.add,
                                )

                if mid is not None:
                    mid()
                # ---- Q projection ----
                qt = pqt.tile([128, 8, QTOK], bf, tag="qt", name="qt")
                for dt in range(8):
                    qacc = pps.tile([128, QTOK], f32, tag="ps", name="qacc")
                    for c in range(8):
                        nc.tensor.matmul(
                            qacc[:, :],
                            wq[:, c, dt * 128 : (dt + 1) * 128],
                            q_sb[:, c, :],
                            start=(c == 0),
                            stop=(c == 7),
                        )
                    nc.vector.tensor_scalar_add(
                        qt[:, dt, :], qacc[:, :], bq_sb[:, dt : dt + 1]
                    )

                if prefetch is not None:
                    prefetch()
                # ---- heads (+ interleaved LN statistic accumulation) ----
                # two heads (sharing a kt dtile) are interleaved per j-chunk
                # so the PE always has ~8 matmuls queued per exp window and
                # never idles into a lower p-state waiting on the scalar exp.
                stats = ppp.tile([128, QTOK], f32, tag="acc", name="stats")
                for hp in range(8):
                    dt = hp
                    g = dt // 2
                    pcs = []
                    ets = [[], []]
                    for par in range(2):
                        pcs.append(ppc.tile([128, QTOK], f32, tag="pc",
                                            name=f"pc{par}"))
                    for jg in range(9):
                        if jg < 8:
                            for par in range(2):
                                ps = pps.tile([128, 2, QTOK], f32, tag="ps",
                                              name="ps")
                                for js in range(2):
                                    j = jg * 2 + js
                                    nc.tensor.matmul(
                                        ps[:, js, :],
                                        kts[dt][par * 64 : par * 64 + 64,
                                                j * 128 : (j + 1) * 128],
                                        qt[par * 64 : par * 64 + 64, dt, :],
                                        start=True,
                                        stop=True,
                                    )
                                et = pexp.tile([128, 2, QTOK], bf, tag="et",
                                               name="et")
                                nc.scalar.activation(
                                    out=et, in_=ps, func=AF.Exp,
                                    scale=1.0 / (HD ** 0.5),
                                )
                                ets[par].append(et)
                        if jg >= 1:
                            jj = jg - 1
                            for par in range(2):
                                h = 2 * hp + par
                                vlo = (h % 4) * 65 + 1
                                for js in range(2):
                                    j = jj * 2 + js
                                    nc.tensor.matmul(
                                        pcs[par][0:65, :],
                                        vp[:, j, g, vlo : vlo + 65],
                                        ets[par][jj][:, js, :],
                                        start=(jj == 0 and js == 0),
                                        stop=(jj == 7 and js == 1),
                                    )
                    # normalize both heads: reciprocal of denom (psum row
                    # 64), broadcast, multiply (DVE has no divide op)
                    for par in range(2):
                        pc = pcs[par]
                        dn = psq.tile([128, QTOK], f32, tag="sq", name="dn")
                        nc.vector.tensor_copy(out=dn[64:65, :],
                                              in_=pc[64:65, :])
                        nc.gpsimd.dma_start(out=dn[0:1, :], in_=dn[64:65, :])
                        dn2 = psq.tile([128, QTOK], f32, tag="sq",
                                       name="dn2")
                        nc.vector.reciprocal_approx_fast(out=dn2[0:1, :],
                                                         in_=dn[0:1, :])
                        db = pexp.tile([128, QTOK], f32, tag="db", name="db",
                                       bufs=2)
                        nc.gpsimd.partition_broadcast(
                            out_ap=db[0:64, :], in_ap=dn2[0:64, :], channels=64
                        )
                        if par == 0:
                            nc.vector.tensor_tensor(
                                out=ctx_tile[0:64, dt, :],
                                in0=pc[0:64, :],
                                in1=db[0:64, :],
                                op=Alu.mult,
                            )
                        else:
                            cn = psq.tile([128, QTOK], bf, tag="sq",
                                          name="cn")
                            nc.vector.tensor_tensor(
                                out=cn[0:64, :],
                                in0=pc[0:64, :],
                                in1=db[0:64, :],
                                op=Alu.mult,
                            )
                            nc.gpsimd.dma_start(
                                out=ctx_tile[64:128, dt, :], in_=cn[0:64, :]
                            )
                # residual adds + LN stats, deferred out of the head loop
                # so the stat matmuls never block the tensor stream on a
                # normalize chain mid-heads; only the last pair's chain is
                # exposed, once.
                for i in range(8):
                    nc.vector.tensor_tensor(
                        out=resid[:, i, :], in0=resid[:, i, :],
                        in1=ctx_tile[:, i, :], op=Alu.add,
                    )
                    nc.tensor.matmul(
                        stats[0:1, :], ones[:, 0:1], resid[:, i, :],
                        start=(i == 0), stop=(i == 7),
                    )
                    sqt = psq.tile([128, QTOK], bf, tag="sq", name="sqt")
                    nc.vector.tensor_tensor(
                        out=sqt, in0=resid[:, i, :], in1=resid[:, i, :],
                        op=Alu.mult,
                    )
                    nc.tensor.matmul(
                        stats[32:33, :], ones[:, 0:1], sqt[:, :],
                        start=(i == 0), stop=(i == 7),
                    )
                return stats

            def ln_finish(stats, gname, x_in, out_tile, out_dma=False):
                """Broadcast stats, then out = LN(x_in) * g + b per tile."""
                g_sb, b_sb = sb[f"{gname}_g"], sb[f"{gname}_b"]
                s_sb = pst.tile([128, 2, QTOK], f32, tag="sb", name="s_sb")
                nc.vector.tensor_copy(out=s_sb[0:1, 0, :], in_=stats[0:1, :])
                nc.vector.tensor_copy(out=s_sb[32:33, 1, :],
                                      in_=stats[32:33, :])
                nc.gpsimd.dma_start(out=s_sb[0:1, 1, :],
                                    in_=s_sb[32:33, 1, :])
                mraw = pst.tile([128, QTOK], f32, tag="meanF", name="mraw")
                nc.gpsimd.partition_broadcast(
                    out_ap=mraw[:, :], in_ap=s_sb[:, 0, :], channels=128
                )
                meanb = pst.tile([128, QTOK], bf, tag="mean", name="meanb")
                nc.vector.tensor_scalar_mul(meanb, mraw, 1.0 / D)
                nc.vector.tensor_scalar_mul(mraw, mraw, 1.0 / D)
                varb = pst.tile([128, QTOK], f32, tag="var", name="varb")
                nc.gpsimd.partition_broadcast(
                    out_ap=varb[:, :], in_ap=s_sb[:, 1, :], channels=128
                )
                nc.vector.tensor_scalar_mul(varb, varb, 1.0 / D)
                nc.vector.tensor_mul(mraw, mraw, mraw)
                nc.vector.tensor_sub(varb, varb, mraw)
                nc.vector.tensor_scalar_add(varb, varb, EPS)
                vrc = pst.tile([128, QTOK], f32, tag="vrc", name="vrc")
                nc.vector.reciprocal_approx_fast(out=vrc, in_=varb)
                rstdb = pst.tile([128, QTOK], bf, tag="rstd", name="rstdb")
                nc.scalar.activation(out=rstdb, in_=vrc, func=AF.Sqrt)
                for i in range(8):
                    t1 = psq.tile([128, QTOK], bf, tag="sq", name="t1")
                    nc.vector.tensor_sub(t1, x_in[:, i, :], meanb)
                    nc.vector.tensor_mul(t1, t1, rstdb)
                    if out_dma:
                        if i % 2 == 0:
                            o = pout.tile([128, QTOK], f32, tag="o", name="o")
                        else:
                            o = mraw
                        nc.vector.tensor_scalar(
                            out=o, in0=t1,
                            scalar1=g_sb[:, i : i + 1], scalar2=b_sb[:, i : i + 1],
                            op0=Alu.mult, op1=Alu.add,
                        )
                        nc.sync.dma_start(out=outT_r[:, i, :], in_=o)
                    else:
                        nc.vector.tensor_scalar(
                            out=out_tile[:, i, :], in0=t1,
                            scalar1=g_sb[:, i : i + 1], scalar2=b_sb[:, i : i + 1],
                            op0=Alu.mult, op1=Alu.add,
                        )

            def eight_psums():
                a = ppp.tile([128, QTOK], f32, tag="acc", name="fa0")
                b = [ppc.tile([128, QTOK], f32, tag="pc", name=f"fb{_i}")
                     for _i in range(3)]
                c_ = [pps.tile([128, 2, QTOK], f32, tag="ps", name=f"fc{_i}")
                      for _i in range(2)]
                return [a[:, :], b[0][:, :], b[1][:, :], b[2][:, :],
                        c_[0][:, 0, :], c_[0][:, 1, :],
                        c_[1][:, 0, :], c_[1][:, 1, :]]

            # ================= self-attention =================
            ca_w = {}
            enc_xb = []

            def prefetch_ca():
                for wn in ("wk", "wv", "wq"):
                    t = pw.tile([128, 8, D], bf, tag="w", name=f"ca_{wn}")
                    nc.sync.dma_start(out=t, in_=w[f"ca_{wn}"][:, :, :])
                    ca_w[wn] = t
                for blk in range(2):
                    t = pxb.tile([128, 8, QTOK], bf, tag="xb", name="exb")
                    nc.gpsimd.dma_start(
                        out=t, in_=encT[:, :, blk * 512 : (blk + 1) * 512]
                    )
                    enc_xb.append(t)

            ffn_w = {}

            def prefetch_ffn():
                for nm, hnd, sl in (
                    ("f1s0", fc1_w, (slice(None), slice(None), slice(0, D))),
                    ("f2s0", fc2_w, (slice(None), slice(0, 8), slice(None))),
                    ("f1s1", fc1_w, (slice(None), slice(None), slice(D, 2 * D))),
                ):
                    t = pw.tile([128, 8, D], bf, tag="w", name=nm)
                    nc.sync.dma_start(out=t, in_=hnd[sl])
                    ffn_w[nm] = t

            ctx1 = pctx.tile([128, 8, QTOK], bf, tag="ctx", name="ctx1")
            stats1 = attention("sa", xT, xq_sb, ctx1, xq_sb,
                               prefetch=prefetch_ca)
            x2 = pxa.tile([128, 8, QTOK], bf, tag="xa", name="x2")

            # ================= cross-attention =================
            # LN1 is emitted as `mid` inside attention("ca"): its DVE chain
            # runs while the PE streams CA's K/V projections (enc-based, no
            # dependency on x2), instead of blocking their PSUM drains.
            ctx2 = pctx.tile([128, 8, QTOK], bf, tag="ctx", name="ctx2")
            stats2 = attention("ca", encT, x2, ctx2, x2,
                               prefetch=prefetch_ffn, pre_xb=enc_xb,
                               pre_w=ca_w,
                               mid=lambda: ln_finish(stats1, "ln1",
                                                     xq_sb, x2))
            x3 = pqt.tile([128, 8, QTOK], bf, tag="qt", name="x3")
            ln_finish(stats2, "ln2", x2, x3)

            # ================= feed-forward =================
            h2 = ph2.tile([128, 8, QTOK], bf, tag="h2", name="h2")
            for qtr in range(4):
                if qtr == 0:
                    f1s = ffn_w["f1s0"]
                elif qtr == 1:
                    f1s = ffn_w["f1s1"]
                else:
                    f1s = pw.tile([128, 8, D], bf, tag="w", name="f1s")
                    nc.sync.dma_start(
                        out=f1s, in_=fc1_w[:, :, qtr * D : (qtr + 1) * D]
                    )
                h1a = eight_psums()
                h1q = pctx.tile([128, 8, QTOK], bf, tag="ctx", name="h1q")
                if qtr == 0:
                    # c-outer: consume x3 chunks as LN2 emits them
                    for c in range(8):
                        for f in range(8):
                            nc.tensor.matmul(
                                h1a[f],
                                f1s[:, c, f * 128 : (f + 1) * 128],
                                x3[:, c, :],
                                start=(c == 0),
                                stop=(c == 7),
                            )
                    for f in range(8):
                        nc.scalar.activation(
                            out=h1q[:, f, :], in_=h1a[f], func=AF.Relu,
                            bias=fc1b_sb[:, qtr * 8 + f : qtr * 8 + f + 1],
                        )
                else:
                    for f in range(8):
                        for c in range(8):
                            nc.tensor.matmul(
                                h1a[f],
                                f1s[:, c, f * 128 : (f + 1) * 128],
                                x3[:, c, :],
                                start=(c == 0),
                                stop=(c == 7),
                            )
                        nc.scalar.activation(
                            out=h1q[:, f, :], in_=h1a[f], func=AF.Relu,
                            bias=fc1b_sb[:, qtr * 8 + f : qtr * 8 + f + 1],
                        )
                if qtr == 0:
                    f2s = ffn_w["f2s0"]
                else:
                    f2s = pw.tile([128, 8, D], bf, tag="w", name="f2s")
                    nc.sync.dma_start(
                        out=f2s, in_=fc2_w[:, qtr * 8 : (qtr + 1) * 8, :]
                    )
                h2a = eight_psums()
                for i in range(8):
                    for f in range(8):
                        nc.tensor.matmul(
                            h2a[i],
                            f2s[:, f, i * 128 : (i + 1) * 128],
                            h1q[:, f, :],
                            start=(f == 0),
                            stop=(f == 7),
                        )
                    if qtr == 0:
                        nc.vector.tensor_copy(out=h2[:, i, :], in_=h2a[i])
                    else:
                        hc = psq.tile([128, QTOK], bf, tag="sq", name="hc")
                        nc.vector.tensor_copy(out=hc, in_=h2a[i])
                        nc.vector.tensor_tensor(
                            out=h2[:, i, :], in0=h2[:, i, :], in1=hc[:, :],
                            op=Alu.add,
                        )

            # residual + LN3 (stats interleaved per tile) + output DMA
            stats3 = ppp.tile([128, QTOK], f32, tag="acc", name="stats3")
            for i in range(8):
                nc.vector.tensor_scalar_add(
                    h2[:, i, :], h2[:, i, :], fc2b_sb[:, i : i + 1]
                )
                nc.vector.tensor_tensor(
                    out=x3[:, i, :], in0=x3[:, i, :], in1=h2[:, i, :],
                    op=Alu.add,
                )
                nc.tensor.matmul(
                    stats3[0:1, :], ones[:, 0:1], x3[:, i, :],
                    start=(i == 0), stop=(i == 7),
                )
                sqt = psq.tile([128, QTOK], bf, tag="sq", name="sqt3")
                nc.vector.tensor_tensor(
                    out=sqt, in0=x3[:, i, :], in1=x3[:, i, :], op=Alu.mult
                )
                nc.tensor.matmul(
                    stats3[32:33, :], ones[:, 0:1], sqt[:, :],
                    start=(i == 0), stop=(i == 7),
                )
            ln_finish(stats3, "ln3", x3, None, out_dma=True)

    nc.compile()
    return nc


def _get_program(mode=None):
    key = "bf16"
    if key not in _PROGRAM_CACHE:
        _PROGRAM_CACHE[key] = _build_program()
    return _PROGRAM_CACHE[key]


def _make_in_maps(inputs):
    import ml_dtypes

    f = np.float32
    bf = ml_dtypes.bfloat16

    def cpn(a, c):  # [c*128, N] -> [128, c, N]
        a = np.asarray(a, dtype=f)
        return np.ascontiguousarray(
            a.reshape(c, 128, a.shape[1]).transpose(1, 0, 2).astype(bf)
        )

    def colmajor8(v):
        return np.ascontiguousarray(np.asarray(v).reshape(8, 128).T.astype(f))

    shared = {}
    for p in ("sa", "ca"):
        for wn in ("wq", "wk", "wv"):
            shared[f"{p}_{wn}"] = cpn(inputs[f"{p}_{wn}"], 8)
        for bn in ("bq", "bk"):
            shared[f"{p}_{bn}"] = colmajor8(inputs[f"{p}_{bn}"])
        shared[f"{p}_bv"] = np.ascontiguousarray(
            np.broadcast_to(np.asarray(inputs[f"{p}_bv"], dtype=f), (128, D))
        ).astype(bf)
    shared["fc1_w"] = cpn(inputs["fc1_w"], 8)
    shared["fc2_w"] = cpn(inputs["fc2_w"], 32)
    shared["fc1_b"] = np.ascontiguousarray(
        np.asarray(inputs["fc1_b"]).reshape(32, 128).T.astype(f)
    )
    shared["fc2_b"] = colmajor8(inputs["fc2_b"])
    for i in (1, 2, 3):
        shared[f"ln{i}_g"] = colmajor8(inputs[f"ln{i}_g"])
        shared[f"ln{i}_b"] = colmajor8(inputs[f"ln{i}_b"])

    hs = np.asarray(inputs["hidden_states"], dtype=f)
    enc = np.asarray(inputs["encoder_hidden_states"], dtype=f)
    in_maps = []
    xt_c = {}
    enc_c = {}
    for b in range(2):
        xt_c[b] = cpn(np.ascontiguousarray(hs[b].T), 8)
        enc_c[b] = cpn(np.ascontiguousarray(enc[b].T), 8)
    for c in range(NCORES):
        b, q0 = c // 4, (c % 4) * QTOK
        m = dict(shared)
        m["xT"] = xt_c[b]
        m["xqT"] = np.ascontiguousarray(xt_c[b][:, :, q0 : q0 + QTOK])
        m["encT"] = enc_c[b]
        in_maps.append(m)
    return in_maps


def kernel(**inputs):
    from concourse.bass_utils import run_bass_kernel_spmd

    nc = _get_program()
    in_maps = _make_in_maps(inputs)
    res = run_bass_kernel_spmd(nc, in_maps, core_ids=list(range(NCORES)))
    out = np.empty((2, S, D), np.float32)
    for c in range(NCORES):
        b, q0 = c // 4, (c % 4) * QTOK
        out[b, q0 : q0 + QTOK, :] = res.results[c]["outT"].T
    return out



# revision 10
# speedup vs baseline: 1.1837x; 1.0046x over previous
"""Trainium2 Bass kernel for nn_DecoderLayer (self-attn + cross-attn + FFN).

Sharding: 8 cores = 2 batches x 4 query-blocks of 512 tokens (data/sequence
parallel, zero collectives). Each core recomputes the full K/V for its batch
and runs its 512 queries through the whole layer.

v2: all matmul operands bf16 (weights/activations cast host-side where
possible) -- same 1 cyc/row PE rate as f32r but half the DMA bytes and half
the SBUF footprint. K/V for all 4 head-groups stay resident so X/enc stream
exactly once per attention. Per-attention order is K/V proj -> Q proj ->
heads, so cross-attention K/V (which needs only encT) overlaps LN1's vector
chain. Two heads sharing a kt dtile are interleaved per j-chunk so the PE
stays fed through the scalar-engine exp latency; residual adds + LayerNorm
statistics are deferred to one block after the head loop so the stat
matmuls never stall the tensor stream on a softmax-normalize chain.
Next-phase weights (CA, FFN slabs) and encoder blocks are prefetched
before each head loop.
"""

import sys

if "/opt/trn_rl_repo" not in sys.path:
    sys.path.insert(0, "/opt/trn_rl_repo")

import numpy as np

D = 1024
S = 2048
QTOK = 512          # queries per core
H = 16
HD = 64
FFN = 4096
EPS = 1e-5
NCORES = 8

_PROGRAM_CACHE = {}


def _build_program():
    import contextlib

    import concourse.bacc as bacc
    import concourse.tile as tile
    from concourse import mybir

    f32 = mybir.dt.float32
    bf = mybir.dt.bfloat16
    AF = mybir.ActivationFunctionType
    Alu = mybir.AluOpType

    nc = bacc.Bacc("TRN2", target_bir_lowering=False)

    def din(name, shape, dt=bf):
        return nc.declare_dram_parameter(name, list(shape), dt, isOutput=False)

    xT = din("xT", (128, 8, S))
    xqT = din("xqT", (128, 8, QTOK))
    encT = din("encT", (128, 8, S))
    w = {}
    for p in ("sa", "ca"):
        for wn in ("wq", "wk", "wv"):
            w[f"{p}_{wn}"] = din(f"{p}_{wn}", (128, 8, D))
        for bn in ("bq", "bk"):
            w[f"{p}_{bn}"] = din(f"{p}_{bn}", (128, 8), f32)
        w[f"{p}_bv"] = din(f"{p}_bv", (128, D))
    fc1_w = din("fc1_w", (128, 8, FFN))
    fc2_w = din("fc2_w", (128, 32, D))
    fc1_b = din("fc1_b", (128, 32), f32)
    fc2_b = din("fc2_b", (128, 8), f32)
    for i in (1, 2, 3):
        w[f"ln{i}_g"] = din(f"ln{i}_g", (128, 8), f32)
        w[f"ln{i}_b"] = din(f"ln{i}_b", (128, 8), f32)
    outT = nc.declare_dram_parameter("outT", [D, QTOK], f32, isOutput=True)
    outT_r = outT.rearrange("(i p) t -> p i t", p=128)

    with tile.TileContext(nc) as tc:
        with contextlib.ExitStack() as ctx:
            consts = ctx.enter_context(tc.tile_pool(name="consts", bufs=1))
            pw = ctx.enter_context(tc.tile_pool(name="pw", bufs=3))
            pkt = ctx.enter_context(tc.tile_pool(name="pkt", bufs=8))
            pvp = ctx.enter_context(tc.tile_pool(name="pvp", bufs=1))
            pqt = ctx.enter_context(tc.tile_pool(name="pqt", bufs=1))
            pxb = ctx.enter_context(tc.tile_pool(name="pxb", bufs=2))
            pctx = ctx.enter_context(tc.tile_pool(name="pctx", bufs=1))
            pxa = ctx.enter_context(tc.tile_pool(name="pxa", bufs=1))
            ph2 = ctx.enter_context(tc.tile_pool(name="ph2", bufs=1))
            pst = ctx.enter_context(tc.tile_pool(name="pst", bufs=1))
            psq = ctx.enter_context(tc.tile_pool(name="psq", bufs=2))
            pexp = ctx.enter_context(tc.tile_pool(name="pexp", bufs=4))
            pout = ctx.enter_context(tc.tile_pool(name="pout", bufs=1))
            ppp = ctx.enter_context(tc.tile_pool(name="ppp", bufs=1, space="PSUM"))
            pps = ctx.enter_context(tc.tile_pool(name="pps", bufs=2, space="PSUM"))
            ppc = ctx.enter_context(tc.tile_pool(name="ppc", bufs=3, space="PSUM"))

            # ---- constants ----
            ones = consts.tile([128, 1], bf, tag="ones")
            nc.vector.memset(ones, 1.0)
            xq_sb = consts.tile([128, 8, QTOK], bf, tag="xq")
            nc.gpsimd.dma_start(out=xq_sb, in_=xqT[:, :, :])
            sb = {}
            for name, hnd in w.items():
                if name.endswith(("wq", "wk", "wv")):
                    continue
                dt_ = bf if name.endswith("_bv") else f32
                sb[name] = consts.tile(
                    [128, hnd.shape[1]], dt_, tag=name, name=name
                )
                nc.scalar.dma_start(out=sb[name], in_=hnd[:, :])
            fc1b_sb = consts.tile([128, 32], f32, tag="fc1b")
            nc.scalar.dma_start(out=fc1b_sb, in_=fc1_b[:, :])
            fc2b_sb = consts.tile([128, 8], f32, tag="fc2b")
            nc.scalar.dma_start(out=fc2b_sb, in_=fc2_b[:, :])

            def attention(pfx, src_dram, q_sb, ctx_tile, resid,
                          prefetch=None, pre_xb=None, pre_w=None, mid=None):
                """One MHA. K/V proj (streaming src once) -> Q proj -> heads.
                After each head pair, the residual add + LN stat matmuls for
                that dtile are emitted (accumulating into sum_ps/sq_ps, which
                are allocated last on the 'acc' ring so nothing displaces
                them mid-accumulation). Returns (sum_ps, sq_ps)."""
                bq_sb, bk_sb, bv_sb = sb[f"{pfx}_bq"], sb[f"{pfx}_bk"], sb[f"{pfx}_bv"]
                if pre_w is not None:
                    wk, wv, wq = pre_w["wk"], pre_w["wv"], pre_w["wq"]
                else:
                    wk = pw.tile([128, 8, D], bf, tag="w", name="wk")
                    nc.sync.dma_start(out=wk, in_=w[f"{pfx}_wk"][:, :, :])
                    wv = pw.tile([128, 8, D], bf, tag="w", name="wv")
                    nc.sync.dma_start(out=wv, in_=w[f"{pfx}_wv"][:, :, :])
                    wq = pw.tile([128, 8, D], bf, tag="w", name="wq")
                    nc.sync.dma_start(out=wq, in_=w[f"{pfx}_wq"][:, :, :])

                kts = [
                    pkt.tile([128, S], bf, tag="kt", name=f"kt{_d}")
                    for _d in range(8)
                ]
                vp = pvp.tile([128, 16, 4, 261], bf, tag="vp", name="vp")
                # ones at col h*65 of each group block, plus trailing col 260
                nc.vector.memset(
                    vp[:, :, :, 0 : 65 * 4].rearrange(
                        "p j g (h u) -> p j g h u", u=65
                    )[:, :, :, :, 0:1],
                    1.0,
                )
                nc.vector.memset(vp[:, :, :, 260:261], 1.0)

                # ---- K/V projections, one pass over src ----
                for blk in range(4):
                    if pre_xb is not None and blk < len(pre_xb):
                        xb = pre_xb[blk]
                    else:
                        xb = pxb.tile([128, 8, QTOK], bf, tag="xb", name="xb")
                        nc.gpsimd.dma_start(
                            out=xb,
                            in_=src_dram[:, :, blk * 512 : (blk + 1) * 512],
                        )
                    for dt in range(8):
                        kacc = pps.tile([128, QTOK], f32, tag="ps", name="kacc")
                        for c in range(8):
                            nc.tensor.matmul(
                                kacc[:, :],
                                wk[:, c, dt * 128 : (dt + 1) * 128],
                                xb[:, c, :],
                                start=(c == 0),
                                stop=(c == 7),
                            )
                        nc.vector.tensor_scalar_add(
                            kts[dt][:, blk * 512 : (blk + 1) * 512],
                            kacc[:, :],
                            bk_sb[:, dt : dt + 1],
                        )
                    for tt in range(4):
                        j = blk * 4 + tt
                        for hf in range(2):
                            vacc = pps.tile([128, QTOK], f32, tag="ps",
                                            name="vacc")
                            for c in range(8):
                                nc.tensor.matmul(
                                    vacc[:, :],
                                    xb[:, c, tt * 128 : (tt + 1) * 128],
                                    wv[:, c, hf * 512 : (hf + 1) * 512],
                                    start=(c == 0),
                                    stop=(c == 7),
                                )
                            vc = psq.tile([128, QTOK], bf, tag="sq",
                                          name="vc")
                            nc.vector.tensor_copy(out=vc, in_=vacc[:, :])
                            for gg in range(2):
                                g = hf * 2 + gg
                                dst = vp[:, j, g, 1:261].rearrange(
                                    "p (h u) -> p h u", u=65
                                )[:, :, 0:64]
                                nc.vector.tensor_tensor(
                                    out=dst,
                                    in0=vc[:, gg * 256 : (gg + 1) * 256]
                                    .rearrange("p (h u) -> p h u", u=64),
                                    in1=bv_sb[:, g * 256 : (g + 1) * 256]
                                    .rearrange("p (h u) -> p h u", u=64),
                                    op=Alu.add,
                                )

                if mid is not None:
                    mid()
                # ---- Q projection ----
                qt = pqt.tile([128, 8, QTOK], bf, tag="qt", name="qt")
                for dt in range(8):
                    qacc = pps.tile([128, QTOK], f32, tag="ps", name="qacc")
                    for c in range(8):
                        nc.tensor.matmul(
                            qacc[:, :],
                            wq[:, c, dt * 128 : (dt + 1) * 128],
                            q_sb[:, c, :],
                            start=(c == 0),
                            stop=(c == 7),
                        )
                    nc.vector.tensor_scalar_add(
                        qt[:, dt, :], qacc[:, :], bq_sb[:, dt : dt + 1]
                    )

                if prefetch is not None:
                    prefetch()
                # ---- heads (+ interleaved LN statistic accumulation) ----
                # two heads (sharing a kt dtile) are interleaved per j-chunk
                # so the PE always has ~8 matmuls queued per exp window and
                # never idles into a lower p-state waiting on the scalar exp.
                stats = ppp.tile([128, QTOK], f32, tag="acc", name="stats")
                for hp in range(8):
                    dt = hp
                    g = dt // 2
                    pcs = []
                    ets = [[], []]
                    for par in range(2):
                        pcs.append(ppc.tile([128, QTOK], f32, tag="pc",
                                            name=f"pc{par}"))
                    for jg in range(9):
                        if jg < 8:
                            for par in range(2):
                                ps = pps.tile([128, 2, QTOK], f32, tag="ps",
                                              name="ps")
                                for js in range(2):
                                    j = jg * 2 + js
                                    nc.tensor.matmul(
                                        ps[:, js, :],
                                        kts[dt][par * 64 : par * 64 + 64,
                                                j * 128 : (j + 1) * 128],
                                        qt[par * 64 : par * 64 + 64, dt, :],
                                        start=True,
                                        stop=True,
                                    )
                                et = pexp.tile([128, 2, QTOK], bf, tag="et",
                                               name="et")
                                nc.scalar.activation(
                                    out=et, in_=ps, func=AF.Exp,
                                    scale=1.0 / (HD ** 0.5),
                                )
                                ets[par].append(et)
                        if jg >= 1:
                            jj = jg - 1
                            for par in range(2):
                                h = 2 * hp + par
                                vlo = (h % 4) * 65 + 1
                                for js in range(2):
                                    j = jj * 2 + js
                                    nc.tensor.matmul(
                                        pcs[par][0:65, :],
                                        vp[:, j, g, vlo : vlo + 65],
                                        ets[par][jj][:, js, :],
                                        start=(jj == 0 and js == 0),
                                        stop=(jj == 7 and js == 1),
                                    )
                    # normalize both heads: reciprocal of denom (psum row
                    # 64), broadcast, multiply (DVE has no divide op)
                    for par in range(2):
                        pc = pcs[par]
                        dn = psq.tile([128, QTOK], f32, tag="sq", name="dn")
                        nc.vector.tensor_copy(out=dn[64:65, :],
                                              in_=pc[64:65, :])
                        nc.gpsimd.dma_start(out=dn[0:1, :], in_=dn[64:65, :])
                        dn2 = psq.tile([128, QTOK], f32, tag="sq",
                                       name="dn2")
                        nc.vector.reciprocal_approx_fast(out=dn2[0:1, :],
                                                         in_=dn[0:1, :])
                        db = pexp.tile([128, QTOK], f32, tag="db", name="db",
                                       bufs=4)
                        nc.gpsimd.partition_broadcast(
                            out_ap=db[0:64, :], in_ap=dn2[0:64, :], channels=64
                        )
                        if par == 0:
                            nc.vector.tensor_tensor(
                                out=ctx_tile[0:64, dt, :],
                                in0=pc[0:64, :],
                                in1=db[0:64, :],
                                op=Alu.mult,
                            )
                        else:
                            cn = psq.tile([128, QTOK], bf, tag="sq",
                                          name="cn")
                            nc.vector.tensor_tensor(
                                out=cn[0:64, :],
                                in0=pc[0:64, :],
                                in1=db[0:64, :],
                                op=Alu.mult,
                            )
                            nc.gpsimd.dma_start(
                                out=ctx_tile[64:128, dt, :], in_=cn[0:64, :]
                            )
                # residual adds + LN stats, deferred out of the head loop
                # so the stat matmuls never block the tensor stream on a
                # normalize chain mid-heads; only the last pair's chain is
                # exposed, once.
                for i in range(8):
                    nc.vector.tensor_tensor(
                        out=resid[:, i, :], in0=resid[:, i, :],
                        in1=ctx_tile[:, i, :], op=Alu.add,
                    )
                    nc.tensor.matmul(
                        stats[0:1, :], ones[:, 0:1], resid[:, i, :],
                        start=(i == 0), stop=(i == 7),
                    )
                    sqt = psq.tile([128, QTOK], bf, tag="sq", name="sqt")
                    nc.vector.tensor_tensor(
                        out=sqt, in0=resid[:, i, :], in1=resid[:, i, :],
                        op=Alu.mult,
                    )
                    nc.tensor.matmul(
                        stats[32:33, :], ones[:, 0:1], sqt[:, :],
                        start=(i == 0), stop=(i == 7),
                    )
                return stats

            def ln_finish(stats, gname, x_in, out_tile, out_dma=False):
                """Broadcast stats, then out = LN(x_in) * g + b per tile."""
                g_sb, b_sb = sb[f"{gname}_g"], sb[f"{gname}_b"]
                s_sb = pst.tile([128, 2, QTOK], f32, tag="sb", name="s_sb")
                nc.vector.tensor_copy(out=s_sb[0:1, 0, :], in_=stats[0:1, :])
                nc.vector.tensor_copy(out=s_sb[32:33, 1, :],
                                      in_=stats[32:33, :])
                nc.gpsimd.dma_start(out=s_sb[0:1, 1, :],
                                    in_=s_sb[32:33, 1, :])
                mraw = pst.tile([128, QTOK], f32, tag="meanF", name="mraw")
                nc.gpsimd.partition_broadcast(
                    out_ap=mraw[:, :], in_ap=s_sb[:, 0, :], channels=128
                )
                meanb = pst.tile([128, QTOK], bf, tag="mean", name="meanb")
                nc.vector.tensor_scalar_mul(meanb, mraw, 1.0 / D)
                nc.vector.tensor_scalar_mul(mraw, mraw, 1.0 / D)
                varb = pst.tile([128, QTOK], f32, tag="var", name="varb")
                nc.gpsimd.partition_broadcast(
                    out_ap=varb[:, :], in_ap=s_sb[:, 1, :], channels=128
                )
                nc.vector.tensor_scalar_mul(varb, varb, 1.0 / D)
                nc.vector.tensor_mul(mraw, mraw, mraw)
                nc.vector.tensor_sub(varb, varb, mraw)
                nc.vector.tensor_scalar_add(varb, varb, EPS)
                vrc = pst.tile([128, QTOK], f32, tag="vrc", name="vrc")
                nc.vector.reciprocal_approx_fast(out=vrc, in_=varb)
                rstdb = pst.tile([128, QTOK], bf, tag="rstd", name="rstdb")
                nc.scalar.activation(out=rstdb, in_=vrc, func=AF.Sqrt)
                for i in range(8):
                    t1 = psq.tile([128, QTOK], bf, tag="sq", name="t1")
                    nc.vector.tensor_sub(t1, x_in[:, i, :], meanb)
                    nc.vector.tensor_mul(t1, t1, rstdb)
                    if out_dma:
                        if i % 2 == 0:
                            o = pout.tile([128, QTOK], f32, tag="o", name="o")
                        else:
                            o = mraw
                        nc.vector.tensor_scalar(
                            out=o, in0=t1,
                            scalar1=g_sb[:, i : i + 1], scalar2=b_sb[:, i : i + 1],
                            op0=Alu.mult, op1=Alu.add,
                        )
                        nc.sync.dma_start(out=outT_r[:, i, :], in_=o)
                    else:
                        nc.vector.tensor_scalar(
                            out=out_tile[:, i, :], in0=t1,
                            scalar1=g_sb[:, i : i + 1], scalar2=b_sb[:, i : i + 1],
                            op0=Alu.mult, op1=Alu.add,
                        )

            def eight_psums():
                a = ppp.tile([128, QTOK], f32, tag="acc", name="fa0")
                b = [ppc.tile([128, QTOK], f32, tag="pc", name=f"fb{_i}")
                     for _i in range(3)]
                c_ = [pps.tile([128, 2, QTOK], f32, tag="ps", name=f"fc{_i}")
                      for _i in range(2)]
                return [a[:, :], b[0][:, :], b[1][:, :], b[2][:, :],
                        c_[0][:, 0, :], c_[0][:, 1, :],
                        c_[1][:, 0, :], c_[1][:, 1, :]]

            # ================= self-attention =================
            ca_w = {}
            enc_xb = []

            def prefetch_ca():
                for wn in ("wk", "wv", "wq"):
                    t = pw.tile([128, 8, D], bf, tag="w", name=f"ca_{wn}")
                    nc.sync.dma_start(out=t, in_=w[f"ca_{wn}"][:, :, :])
                    ca_w[wn] = t
                for blk in range(2):
                    t = pxb.tile([128, 8, QTOK], bf, tag="xb", name="exb")
                    nc.gpsimd.dma_start(
                        out=t, in_=encT[:, :, blk * 512 : (blk + 1) * 512]
                    )
                    enc_xb.append(t)

            ffn_w = {}

            def prefetch_ffn():
                for nm, hnd, sl in (
                    ("f1s0", fc1_w, (slice(None), slice(None), slice(0, D))),
                    ("f2s0", fc2_w, (slice(None), slice(0, 8), slice(None))),
                    ("f1s1", fc1_w, (slice(None), slice(None), slice(D, 2 * D))),
                ):
                    t = pw.tile([128, 8, D], bf, tag="w", name=nm)
                    nc.sync.dma_start(out=t, in_=hnd[sl])
                    ffn_w[nm] = t

            ctx1 = pctx.tile([128, 8, QTOK], bf, tag="ctx", name="ctx1")
            stats1 = attention("sa", xT, xq_sb, ctx1, xq_sb,
                               prefetch=prefetch_ca)
            x2 = pxa.tile([128, 8, QTOK], bf, tag="xa", name="x2")

            # ================= cross-attention =================
            # LN1 is emitted as `mid` inside attention("ca"): its DVE chain
            # runs while the PE streams CA's K/V projections (enc-based, no
            # dependency on x2), instead of blocking their PSUM drains.
            ctx2 = pctx.tile([128, 8, QTOK], bf, tag="ctx", name="ctx2")
            stats2 = attention("ca", encT, x2, ctx2, x2,
                               prefetch=prefetch_ffn, pre_xb=enc_xb,
                               pre_w=ca_w,
                               mid=lambda: ln_finish(stats1, "ln1",
                                                     xq_sb, x2))
            x3 = pqt.tile([128, 8, QTOK], bf, tag="qt", name="x3")
            ln_finish(stats2, "ln2", x2, x3)

            # ================= feed-forward =================
            h2 = ph2.tile([128, 8, QTOK], bf, tag="h2", name="h2")
            for qtr in range(4):
                if qtr == 0:
                    f1s = ffn_w["f1s0"]
                elif qtr == 1:
                    f1s = ffn_w["f1s1"]
                else:
                    f1s = pw.tile([128, 8, D], bf, tag="w", name="f1s")
                    nc.sync.dma_start(
                        out=f1s, in_=fc1_w[:, :, qtr * D : (qtr + 1) * D]
                    )
                h1a = eight_psums()
                h1q = pctx.tile([128, 8, QTOK], bf, tag="ctx", name="h1q")
                if qtr == 0:
                    # c-outer: consume x3 chunks as LN2 emits them
                    for c in range(8):
                        for f in range(8):
                            nc.tensor.matmul(
                                h1a[f],
                                f1s[:, c, f * 128 : (f + 1) * 128],
                                x3[:, c, :],
                                start=(c == 0),
                                stop=(c == 7),
                            )
                    for f in range(8):
                        nc.scalar.activation(
                            out=h1q[:, f, :], in_=h1a[f], func=AF.Relu,
                            bias=fc1b_sb[:, qtr * 8 + f : qtr * 8 + f + 1],
                        )
                else:
                    for f in range(8):
                        for c in range(8):
                            nc.tensor.matmul(
                                h1a[f],
                                f1s[:, c, f * 128 : (f + 1) * 128],
                                x3[:, c, :],
                                start=(c == 0),
                                stop=(c == 7),
                            )
                        nc.scalar.activation(
                            out=h1q[:, f, :], in_=h1a[f], func=AF.Relu,
                            bias=fc1b_sb[:, qtr * 8 + f : qtr * 8 + f + 1],
                        )
                if qtr == 0:
                    f2s = ffn_w["f2s0"]
                else:
                    f2s = pw.tile([128, 8, D], bf, tag="w", name="f2s")
                    nc.sync.dma_start(
                        out=f2s, in_=fc2_w[:, qtr * 8 : (qtr + 1) * 8, :]
                    )
                h2a = eight_psums()
                for i in range(8):
                    for f in range(8):
                        nc.tensor.matmul(
                            h2a[i],
                            f2s[:, f, i * 128 : (i + 1) * 128],
                            h1q[:, f, :],
                            start=(f == 0),
                            stop=(f == 7),
                        )
                    if qtr == 0:
                        nc.vector.tensor_copy(out=h2[:, i, :], in_=h2a[i])
                    else:
                        hc = psq.tile([128, QTOK], bf, tag="sq", name="hc")
                        nc.vector.tensor_copy(out=hc, in_=h2a[i])
                        nc.vector.tensor_tensor(
                            out=h2[:, i, :], in0=h2[:, i, :], in1=hc[:, :],
                            op=Alu.add,
                        )

            # residual + LN3 (stats interleaved per tile) + output DMA
            stats3 = ppp.tile([128, QTOK], f32, tag="acc", name="stats3")
            for i in range(8):
                nc.vector.tensor_scalar_add(
                    h2[:, i, :], h2[:, i, :], fc2b_sb[:, i : i + 1]
                )
                nc.vector.tensor_tensor(
                    out=x3[:, i, :], in0=x3[:, i, :], in1=h2[:, i, :],
                    op=Alu.add,
                )
                nc.tensor.matmul(
                    stats3[0:1, :], ones[:, 0:1], x3[:, i, :],
                    start=(i == 0), stop=(i == 7),
                )
                sqt = psq.tile([128, QTOK], bf, tag="sq", name="sqt3")
                nc.vector.tensor_tensor(
                    out=sqt, in0=x3[:, i, :], in1=x3[:, i, :], op=Alu.mult
                )
                nc.tensor.matmul(
                    stats3[32:33, :], ones[:, 0:1], sqt[:, :],
                    start=(i == 0), stop=(i == 7),
                )
            ln_finish(stats3, "ln3", x3, None, out_dma=True)

    nc.compile()
    return nc


def _get_program(mode=None):
    key = "bf16"
    if key not in _PROGRAM_CACHE:
        _PROGRAM_CACHE[key] = _build_program()
    return _PROGRAM_CACHE[key]


def _make_in_maps(inputs):
    import ml_dtypes

    f = np.float32
    bf = ml_dtypes.bfloat16

    def cpn(a, c):  # [c*128, N] -> [128, c, N]
        a = np.asarray(a, dtype=f)
        return np.ascontiguousarray(
            a.reshape(c, 128, a.shape[1]).transpose(1, 0, 2).astype(bf)
        )

    def colmajor8(v):
        return np.ascontiguousarray(np.asarray(v).reshape(8, 128).T.astype(f))

    shared = {}
    for p in ("sa", "ca"):
        for wn in ("wq", "wk", "wv"):
            shared[f"{p}_{wn}"] = cpn(inputs[f"{p}_{wn}"], 8)
        for bn in ("bq", "bk"):
            shared[f"{p}_{bn}"] = colmajor8(inputs[f"{p}_{bn}"])
        shared[f"{p}_bv"] = np.ascontiguousarray(
            np.broadcast_to(np.asarray(inputs[f"{p}_bv"], dtype=f), (128, D))
        ).astype(bf)
    shared["fc1_w"] = cpn(inputs["fc1_w"], 8)
    shared["fc2_w"] = cpn(inputs["fc2_w"], 32)
    shared["fc1_b"] = np.ascontiguousarray(
        np.asarray(inputs["fc1_b"]).reshape(32, 128).T.astype(f)
    )
    shared["fc2_b"] = colmajor8(inputs["fc2_b"])
    for i in (1, 2, 3):
        shared[f"ln{i}_g"] = colmajor8(inputs[f"ln{i}_g"])
        shared[f"ln{i}_b"] = colmajor8(inputs[f"ln{i}_b"])

    hs = np.asarray(inputs["hidden_states"], dtype=f)
    enc = np.asarray(inputs["encoder_hidden_states"], dtype=f)
    in_maps = []
    xt_c = {}
    enc_c = {}
    for b in range(2):
        xt_c[b] = cpn(np.ascontiguousarray(hs[b].T), 8)
        enc_c[b] = cpn(np.ascontiguousarray(enc[b].T), 8)
    for c in range(NCORES):
        b, q0 = c // 4, (c % 4) * QTOK
        m = dict(shared)
        m["xT"] = xt_c[b]
        m["xqT"] = np.ascontiguousarray(xt_c[b][:, :, q0 : q0 + QTOK])
        m["encT"] = enc_c[b]
        in_maps.append(m)
    return in_maps


def kernel(**inputs):
    from concourse.bass_utils import run_bass_kernel_spmd

    nc = _get_program()
    in_maps = _make_in_maps(inputs)
    res = run_bass_kernel_spmd(nc, in_maps, core_ids=list(range(NCORES)))
    out = np.empty((2, S, D), np.float32)
    for c in range(NCORES):
        b, q0 = c // 4, (c % 4) * QTOK
        out[b, q0 : q0 + QTOK, :] = res.results[c]["outT"].T
    return out

